# revision 24
# baseline (speedup 1.0000x reference)
"""MoE layer (8 experts, top-2) Trainium2 kernel — fp16/fp8 tiers + capacity.

Expert-parallel (per sharding hint): host computes the tiny router
(logits -> top-2 -> softmax gates) and dispatches tokens to the 8 NeuronCores
by selected expert; core e runs expert e's FFN.  Each expert's token-pairs
are ranked by gate and split into fixed-size tiers (same sizes on every core
so one NEFF serves all 8 SPMD):

  f16 tier (N16=2176, largest gates)  fp16 mm1 + fp16 mm2           cost 1.00
  mid tier (NM=768)                   fp16 mm1 + fp8 DoubleRow mm2  cost 0.75
  fp8 tier (N8=512, smallest gates)   fp8 DR mm1 + fp8 DR mm2       cost 0.50
  beyond SLOTS=3456 (expert capacity) exact fp32 on host (small tail)

The mid tier quantizes only the mm2 inputs (h, w2), sitting slightly below
the f16<->fp8 cost/error^2 mixing chord; tier sizes were tuned against a
host-side ml_dtypes simulator that predicts the measured HW rel-err to
~1e-4 (target ~0.0195 of the 2e-2 budget).

fp8 (e4m3) DoubleRow matmuls run 2x the fp16 PE rate.  All fp8 operands are
scaled into e4m3's normal range (x*16, w1*64, w2*128) to avoid subnormal
quantization loss; the scales are undone exactly via the activation's
input scale (1/1024, power of two) and host-side gate scaling (g/128).
fp16 (not bf16) for everything else makes the high tier's quantization
error negligible, freeing the 2e-2 rel-err budget for bigger fp8 tiers.
Expert overflow past SLOTS (capacity factor SLOTS/4096) is computed exactly
on the host (tens of GFLOP, sub-second) — standard MoE capacity handling,
with recompute instead of token dropping.

Execution order puts the full-fp8 tier FIRST: its PE work needs only ~9MB
of DMA (fp8 weights + tokens), hiding the cold-start weight load; w1q is
split into 8 per-slice tiles so the first matmul only waits on slice 0.
The mid tier runs second (streams w1h, needs only w2q for mm2); the f16
tier last (needs w2h, which loads during the earlier phases).

SBUF keeps w2h(f16) + w1q/w2q(fp8) resident; w1h streams per supertile.
All streamed tensors are pre-permuted on the host into SBUF consumption
order so each DMA moves >=4KB contiguous per partition.  fp8 x tiles alias
the f16 tiles' storage via bitcast.  All matmuls accumulate in fp32 PSUM.
"""

import numpy as np
import ml_dtypes

P = 128
D_MODEL = 1024
D_FF = 4096
NUM_EXPERTS = 8
KT1 = D_MODEL // P   # 8  k-tiles for mm1
KT2 = D_FF // P      # 32 k-tiles for mm2
FFC = D_FF // P      # 32 ff chunks (mm1 output partition tiles)
NH = D_MODEL // 2    # 512, free dim of mm2 matmuls
W1SL = 256           # ff columns per streamed f16-w1 slice
N_SL = D_FF // W1SL  # 16 slices per supertile
W1QSL = 512          # ff columns per fp8-w1 slice
N_QSL = D_FF // W1QSL

N16 = 2176           # f16 tier slots per expert
NM = 768             # mid tier (f16 mm1 + fp8 mm2)
N8 = 512             # full-fp8 tier
NQ = NM + N8         # rows whose mm2 goes through w2q (gates get /128)
SLOTS = N16 + NM + N8  # 3456 = expert capacity; overflow -> exact host

SX = 16.0            # x scale into e4m3
S1 = 64.0            # w1 scale into e4m3
S2 = 128.0           # w2 scale into e4m3 (undone via gates)
ACT_SCALE = 1.0 / (SX * S1)

F16 = np.float16
F8 = ml_dtypes.float8_e4m3

_NC_CACHE: dict = {}
LAST_RESULTS = None  # BassKernelResults of the most recent device run

# Chunk sizes are chosen so weight loads hide behind the stream: fp8 DR
# matmuls are LDWEIGHTS-bound below 512-wide streams, f16 below 256-wide.
CH16 = [512, 512, 512, 384, 256]   # sums to N16
CHM = [512, 256]                   # sums to NM
CH8 = [512]                        # sums to N8
assert sum(CH16) == N16 and sum(CHM) == NM and sum(CH8) == N8


def _build(act="Gelu"):
    import concourse.bacc as bacc
    import concourse.tile as tile
    import concourse.mybir as mybir

    f16 = mybir.dt.float16
    f8 = mybir.dt.float8e4
    f32 = mybir.dt.float32
    DR = mybir.MatmulPerfMode.DoubleRow
    GELU = getattr(mybir.ActivationFunctionType, act)

    nst16 = len(CH16)
    nstm = len(CHM)
    nst8 = len(CH8)

    nc = bacc.Bacc("TRN2", target_bir_lowering=False, debug=False)
    # All inputs are host-side pre-permuted into [partition, ...consumption
    # order...] so every DMA line is long and contiguous.  xT16 holds the
    # f16-tier supertiles then the mid tier's; xT8 the full-fp8 tier's.
    # Device out rows are in rank order: f16 | mid | fp8.
    xT16_d = nc.dram_tensor("xT16", [P, nst16 + nstm, KT1, 512], f16, kind="ExternalInput").ap()
    xT8_d = nc.dram_tensor("xT8", [P, nst8, KT1, 512], f8, kind="ExternalInput").ap()
    w1h_d = nc.dram_tensor("w1h", [P, N_SL, KT1, W1SL], f16, kind="ExternalInput").ap()
    w1q_d = nc.dram_tensor("w1q", [P, N_QSL, KT1, W1QSL], f8, kind="ExternalInput").ap()
    w2h_d = nc.dram_tensor("w2h", [P, KT2, D_MODEL], f16, kind="ExternalInput").ap()
    w2q_d = nc.dram_tensor("w2q", [P, KT2, D_MODEL], f8, kind="ExternalInput").ap()
    b1_d = nc.dram_tensor("b1", [P, FFC], f32, kind="ExternalInput").ap()
    g16_d = nc.dram_tensor("g16", [P, N16 // P], f32, kind="ExternalInput").ap()
    gq_d = nc.dram_tensor("gq", [P, NQ // P], f32, kind="ExternalInput").ap()
    out_d = nc.dram_tensor("out", [SLOTS, D_MODEL], f32, kind="ExternalOutput").ap()

    with tile.TileContext(nc) as tc:
        with (
            tc.tile_pool(name="wpool", bufs=1) as wpool,
            tc.tile_pool(name="w1pool", bufs=6) as w1pool,
            tc.tile_pool(name="xpool", bufs=2) as xpool,
            tc.tile_pool(name="hpool", bufs=1) as hpool,
            tc.tile_pool(name="opool", bufs=3) as opool,
            tc.tile_pool(name="php", bufs=4, space="PSUM") as php,
            tc.tile_pool(name="pyp", bufs=4, space="PSUM") as pyp,
        ):
            # --- prologue: cold-start DMA is per-transfer-latency-bound, so
            # the first matmul's dependencies (x8 supertile + w1q slice 0) are
            # few fat transfers issued ahead of everything else.  w1q lives in
            # 8 per-slice tiles so matmuls unlock slice by slice.
            # Two HWDGE queues (sync + scalar) split the cold-start ramp: the
            # first matmul's two inputs (x8 on sync, w1q slice 0 on scalar)
            # transfer in parallel, and later slices alternate queues.
            xt8s = []
            for st in range(nst8):
                xtA = xpool.tile([P, KT1, 512], f16, tag="xt")
                xt8s.append(xtA[:].bitcast(f8))
                nc.sync.dma_start(xt8s[st][:, :, :512], xT8_d[:, st])
            w1q_sl = []
            for sl in range(N_QSL):
                t = wpool.tile([P, KT1, W1QSL], f8, tag=f"w1q{sl}")
                eng = nc.scalar if sl % 2 == 0 else nc.sync
                eng.dma_start(t[:], w1q_d[:, sl])
                w1q_sl.append(t)
                if sl == 0:
                    # b1 gates the first activation (and through PSUM reuse,
                    # the 5th ffc's matmuls) — land it right after slice 0.
                    b1_sb = wpool.tile([P, FFC], f32, tag="b1")
                    nc.scalar.dma_start(b1_sb[:], b1_d[:])
            gq_sb = wpool.tile([P, NQ // P], f32, tag="gq")
            nc.sync.dma_start(gq_sb[:], gq_d[:])
            g16_sb = wpool.tile([P, N16 // P], f32, tag="g16")
            nc.sync.dma_start(g16_sb[:], g16_d[:])
            w2q_sb = wpool.tile([P, KT2, D_MODEL], f8, tag="w2q")
            for q in range(4):
                nc.sync.dma_start(
                    w2q_sb[:, q * 8 : (q + 1) * 8, :], w2q_d[:, q * 8 : (q + 1) * 8, :]
                )
            def mm1_fp8(xt8, ht_ffc_view, tok_len):
                """DR mm1 on scaled fp8; activation undoes the x/w1 scales."""
                for ffc in range(FFC):
                    ph = php.tile([P, 512], f32, tag="ph")
                    sl, col = divmod(ffc * P, W1QSL)
                    for k2 in range(KT1 // 2):
                        nc.tensor.matmul(
                            ph[:, :tok_len],
                            w1q_sl[sl][:, 2 * k2 : 2 * k2 + 2, col : col + P],
                            xt8[:, 2 * k2 : 2 * k2 + 2, :tok_len],
                            start=(k2 == 0),
                            stop=(k2 == KT1 // 2 - 1),
                            perf_mode=DR,
                        )
                    nc.scalar.activation(
                        ht_ffc_view(ffc), ph[:, :tok_len], GELU,
                        bias=b1_sb[:, ffc : ffc + 1], scale=ACT_SCALE,
                    )

            def mm1_f16(xt, w1sl, ht_ffc_view, tok_len, act_scale=1.0):
                for ffc in range(FFC):
                    ph = php.tile([P, 512], f32, tag="ph")
                    sl, col = divmod(ffc * P, W1SL)
                    for kt in range(KT1):
                        nc.tensor.matmul(
                            ph[:, :tok_len],
                            w1sl[sl][:, kt, col : col + P],
                            xt[:, kt, :tok_len],
                            start=(kt == 0),
                            stop=(kt == KT1 - 1),
                        )
                    nc.scalar.activation(
                        ht_ffc_view(ffc), ph[:, :tok_len], GELU,
                        bias=b1_sb[:, ffc : ffc + 1], scale=act_scale,
                    )

            def mm2_fp8(ht8, col0, tok_len, out_row0, gcol0, drain_split=False):
                """DR mm2 through scaled w2q; gates (pre-divided by S2) undo it."""
                n_mt = tok_len // P
                for mt in range(n_mt):
                    if drain_split and mt == n_mt - 1:
                        for nb in (0, 1):
                            py = pyp.tile([P, NH], f32, tag="py")
                            for k2 in range(KT2 // 2):
                                nc.tensor.matmul(
                                    py,
                                    ht8[:, 2 * k2 : 2 * k2 + 2, col0 + mt * P : col0 + (mt + 1) * P],
                                    w2q_sb[:, 2 * k2 : 2 * k2 + 2, nb * NH : (nb + 1) * NH],
                                    start=(k2 == 0), stop=(k2 == KT2 // 2 - 1),
                                    perf_mode=DR,
                                )
                            ot = opool.tile([P, NH], f32, tag="ot")
                            nc.vector.tensor_scalar_mul(
                                ot[:], py[:], gq_sb[:, gcol0 + mt : gcol0 + mt + 1]
                            )
                            nc.sync.dma_start(
                                out_d[
                                    out_row0 + mt * P : out_row0 + (mt + 1) * P,
                                    nb * NH : (nb + 1) * NH,
                                ],
                                ot[:],
                            )
                        continue
                    py0 = pyp.tile([P, NH], f32, tag="py")
                    py1 = pyp.tile([P, NH], f32, tag="py")
                    for k2 in range(KT2 // 2):
                        lhsT = ht8[:, 2 * k2 : 2 * k2 + 2, col0 + mt * P : col0 + (mt + 1) * P]
                        nc.tensor.matmul(
                            py0, lhsT, w2q_sb[:, 2 * k2 : 2 * k2 + 2, 0:NH],
                            start=(k2 == 0), stop=(k2 == KT2 // 2 - 1), perf_mode=DR,
                        )
                        nc.tensor.matmul(
                            py1, lhsT, w2q_sb[:, 2 * k2 : 2 * k2 + 2, NH:D_MODEL],
                            start=(k2 == 0), stop=(k2 == KT2 // 2 - 1), perf_mode=DR,
                        )
                    for nb, py in ((0, py0), (1, py1)):
                        ot = opool.tile([P, NH], f32, tag="ot")
                        nc.vector.tensor_scalar_mul(
                            ot[:], py[:], gq_sb[:, gcol0 + mt : gcol0 + mt + 1]
                        )
                        nc.sync.dma_start(
                            out_d[
                                out_row0 + mt * P : out_row0 + (mt + 1) * P,
                                nb * NH : (nb + 1) * NH,
                            ],
                            ot[:],
                        )

            def mm2_f16(ht, tok_len, out_row0, gcol0, drain_split=False):
                n_mt = tok_len // P
                for mt in range(n_mt):
                    if drain_split and mt == n_mt - 1:
                        # Kernel-final block: run four quarter-width output
                        # pieces sequentially so each piece's gate-mult +
                        # store overlaps the next piece's matmuls; only the
                        # last 256-col chain trails the final matmul.
                        NQW = D_MODEL // 4
                        for nb in range(4):
                            py = pyp.tile([P, NH], f32, tag="py")
                            for kt in range(KT2):
                                nc.tensor.matmul(
                                    py[:, :NQW], ht[:, kt, mt * P : (mt + 1) * P],
                                    w2h_sb[:, kt, nb * NQW : (nb + 1) * NQW],
                                    start=(kt == 0), stop=(kt == KT2 - 1),
                                )
                            ot = opool.tile([P, NH], f32, tag="ot")
                            nc.vector.tensor_scalar_mul(
                                ot[:, :NQW], py[:, :NQW],
                                g16_sb[:, gcol0 + mt : gcol0 + mt + 1],
                            )
                            nc.sync.dma_start(
                                out_d[
                                    out_row0 + mt * P : out_row0 + (mt + 1) * P,
                                    nb * NQW : (nb + 1) * NQW,
                                ],
                                ot[:, :NQW],
                            )
                        continue
                    py0 = pyp.tile([P, NH], f32, tag="py")
                    py1 = pyp.tile([P, NH], f32, tag="py")
                    for kt in range(KT2):
                        lhsT = ht[:, kt, mt * P : (mt + 1) * P]
                        nc.tensor.matmul(
                            py0, lhsT, w2h_sb[:, kt, 0:NH],
                            start=(kt == 0), stop=(kt == KT2 - 1),
                        )
                        nc.tensor.matmul(
                            py1, lhsT, w2h_sb[:, kt, NH:D_MODEL],
                            start=(kt == 0), stop=(kt == KT2 - 1),
                        )
                    for nb, py in ((0, py0), (1, py1)):
                        ot = opool.tile([P, NH], f32, tag="ot")
                        nc.vector.tensor_scalar_mul(
                            ot[:], py[:], g16_sb[:, gcol0 + mt : gcol0 + mt + 1]
                        )
                        nc.sync.dma_start(
                            out_d[
                                out_row0 + mt * P : out_row0 + (mt + 1) * P,
                                nb * NH : (nb + 1) * NH,
                            ],
                            ot[:],
                        )

            # ---------- full-fp8 tier (DMA-cheap warmup) ----------
            # All supertiles' mm1 run back-to-back into disjoint column
            # ranges of one h tile; their mm2 follow.
            ht_f8 = hpool.tile([P, KT2, 512], f16, tag="ht")
            ht8 = ht_f8[:].bitcast(f8)  # [P, KT2, 1024] view
            c8 = [0]
            for L in CH8:
                c8.append(c8[-1] + L)
            for st in range(nst8):
                mm1_fp8(
                    xt8s[st],
                    lambda ffc, st=st: ht8[:, ffc, c8[st] : c8[st + 1]],
                    CH8[st],
                )

            # mid/f16-tier streams launch behind the fp8 weights; the fp8 mm2
            # below only needs w2q, which is already queued.
            xt0 = xpool.tile([P, KT1, 512], f16, tag="xt")
            nc.sync.dma_start(xt0[:], xT16_d[:, nst16])  # mid supertile 0
            w1sl0 = []
            for kt in range(2):
                t = w1pool.tile([P, KT1, W1SL], f16, tag="w1sl")
                nc.sync.dma_start(t[:], w1h_d[:, kt])
                w1sl0.append(t)
            # w2h rides the scalar queue so the sync queue stays free for the
            # mid/f16 tiers' streamed w1h slices (8MB here once stalled them).
            w2h_sb = wpool.tile([P, KT2, D_MODEL], f16, tag="w2h")
            for q in range(4):
                nc.scalar.dma_start(
                    w2h_sb[:, q * 8 : (q + 1) * 8, :], w2h_d[:, q * 8 : (q + 1) * 8, :]
                )

            for st in range(nst8):
                mm2_fp8(
                    ht8, c8[st], CH8[st],
                    N16 + NM + c8[st], NM // P + c8[st] // P,
                )

            # ---------- mid tier: f16 mm1 -> fp8 h -> DR mm2 ----------
            tokm = 0
            for stm, tok_len in enumerate(CHM):
                if stm == 0:
                    xt = xt0
                    w1sl = list(w1sl0)
                else:
                    xt = xpool.tile([P, KT1, 512], f16, tag="xt")
                    nc.sync.dma_start(xt[:], xT16_d[:, nst16 + stm])
                    w1sl = []
                for sl in range(len(w1sl), N_SL):
                    t = w1pool.tile([P, KT1, W1SL], f16, tag="w1sl")
                    nc.sync.dma_start(t[:], w1h_d[:, sl])
                    w1sl.append(t)
                htm = hpool.tile([P, KT2, 512], f16, tag="ht")
                htm8 = htm[:].bitcast(f8)  # [P, KT2, 1024] view
                col0 = 512 * (stm % 2)
                mm1_f16(xt, w1sl, lambda ffc: htm8[:, ffc, col0 : col0 + tok_len], tok_len)
                mm2_fp8(htm8, col0, tok_len, N16 + tokm, tokm // P)
                tokm += tok_len

            # ---------- f16 tier ----------
            tok0 = 0
            for st, tok_len in enumerate(CH16):
                xt = xpool.tile([P, KT1, 512], f16, tag="xt")
                nc.sync.dma_start(xt[:], xT16_d[:, st])
                w1sl = []
                for sl in range(N_SL):
                    t = w1pool.tile([P, KT1, W1SL], f16, tag="w1sl")
                    nc.sync.dma_start(t[:], w1h_d[:, sl])
                    w1sl.append(t)
                ht = hpool.tile([P, KT2, 512], f16, tag="ht")
                mm1_f16(xt, w1sl, lambda ffc: ht[:, ffc, :tok_len], tok_len)
                mm2_f16(ht, tok_len, tok0, tok0 // P, drain_split=(st == nst16 - 1))
                tok0 += tok_len
    nc.compile()
    return nc


def _get_nc(act="Gelu"):
    if act not in _NC_CACHE:
        _NC_CACHE[act] = _build(act)
    return _NC_CACHE[act]


def _perm_w(w, kt, cols):
    """[kt*P, cols] -> [P, kt, cols] with row = kt*P + p."""
    return np.ascontiguousarray(w.reshape(kt, P, cols).transpose(1, 0, 2))


def _perm_w_sliced(w, kt, n_sl, slw):
    """[kt*P, n_sl*slw] -> [P, n_sl, kt, slw]."""
    return np.ascontiguousarray(w.reshape(kt, P, n_sl, slw).transpose(1, 2, 0, 3))


def _pack_x(xcols, chunks, dtype):
    """[D_MODEL, n] columns -> [P, n_st, KT1, 512] supertile blocks."""
    out = np.zeros((P, len(chunks), KT1, 512), dtype=dtype)
    t0 = 0
    for st, L in enumerate(chunks):
        blk = xcols[:, t0 : t0 + L].reshape(KT1, P, L).transpose(1, 0, 2)
        out[:, st, :, :L] = blk
        t0 += L
    return out


def _erf(v):
    try:
        from scipy.special import erf
        return erf(v)
    except ImportError:
        # Abramowitz & Stegun 7.1.26 (|err| < 1.5e-7), numpy-only fallback.
        s = np.sign(v)
        a = np.abs(v)
        t = 1.0 / (1.0 + 0.3275911 * a)
        y = 1.0 - (
            ((((1.061405429 * t - 1.453152027) * t) + 1.421413741) * t
             - 0.284496736) * t + 0.254829592
        ) * t * np.exp(-a * a)
        return s * y


def _gelu(v):
    return v * 0.5 * (1.0 + _erf(v / np.sqrt(2.0)))


def kernel(x, router_w, router_b, w1, b1, w2, b2):
    from concourse.bass_utils import run_bass_kernel_spmd

    x = np.asarray(x, dtype=np.float32)
    router_w = np.asarray(router_w, dtype=np.float32)
    router_b = np.asarray(router_b, dtype=np.float32)
    w1 = np.asarray(w1, dtype=np.float32)
    b1 = np.asarray(b1, dtype=np.float32)
    w2 = np.asarray(w2, dtype=np.float32)
    b2 = np.asarray(b2, dtype=np.float32)

    B, S, D = x.shape
    T = B * S
    xf = x.reshape(T, D)

    # --- host router: top-2 + softmax gates (tiny: T x D x 8) ---
    logits = xf @ router_w + router_b                      # [T, 8] fp32
    sel0 = np.argmax(logits, axis=1)
    l0 = logits[np.arange(T), sel0]
    masked = logits.copy()
    masked[np.arange(T), sel0] = -np.inf
    sel1 = np.argmax(masked, axis=1)
    l1 = masked[np.arange(T), sel1]
    e1 = np.exp(l1 - l0)
    g0 = 1.0 / (1.0 + e1)
    g1 = e1 / (1.0 + e1)

    # --- dispatch: per expert, rank pairs by gate; tier by rank.
    # Device slots cover ranks [0, SLOTS) in order f16 | mid | fp8; ranks
    # beyond SLOTS (capacity overflow) are computed exactly on the host.
    tiers = []  # per expert: (ids, g) rank-sorted
    for e in range(NUM_EXPERTS):
        ids0 = np.nonzero(sel0 == e)[0]
        ids1 = np.nonzero(sel1 == e)[0]
        ids = np.concatenate([ids0, ids1])
        g = np.concatenate([g0[ids0], g1[ids1]]).astype(np.float32)
        order = np.argsort(-g, kind="stable")
        tiers.append((ids[order], g[order]))

    nc = _get_nc()

    in_maps = []
    for e in range(NUM_EXPERTS):
        ids, g = tiers[e]
        ids_dev, g_dev = ids[:SLOTS], g[:SLOTS]
        n_dev = len(ids_dev)
        x16 = np.zeros((D_MODEL, N16 + NM), dtype=np.float32)
        x16[:, : min(n_dev, N16 + NM)] = xf[ids_dev[: N16 + NM]].T
        x8 = np.zeros((D_MODEL, N8), dtype=np.float32)
        if n_dev > N16 + NM:
            x8[:, : n_dev - (N16 + NM)] = xf[ids_dev[N16 + NM :]].T
        gp = np.zeros((SLOTS,), dtype=np.float32)
        gp[:n_dev] = g_dev
        gp[N16:] /= S2  # rows whose mm2 goes through w2q (scaled by S2)
        in_maps.append(
            {
                "xT16": _pack_x(x16.astype(F16), CH16 + CHM, F16),
                "xT8": _pack_x(
                    np.clip(x8 * SX, -240.0, 240.0).astype(F8), CH8, F8
                ),
                "w1h": _perm_w_sliced(w1[e].astype(F16), KT1, N_SL, W1SL),
                "w1q": _perm_w_sliced((w1[e] * S1).astype(F8), KT1, N_QSL, W1QSL),
                "w2h": _perm_w(w2[e].astype(F16), KT2, D_MODEL),
                "w2q": _perm_w((w2[e] * S2).astype(F8), KT2, D_MODEL),
                "b1": np.ascontiguousarray(b1[e].reshape(FFC, P).T),
                "g16": np.ascontiguousarray(gp[:N16].reshape(N16 // P, P).T),
                "gq": np.ascontiguousarray(gp[N16:].reshape(NQ // P, P).T),
            }
        )

    try:
        res = run_bass_kernel_spmd(nc, in_maps, core_ids=list(range(NUM_EXPERTS)))
    except Exception:
        # Transient device errors (e.g. NRT_EXEC_UNIT_UNRECOVERABLE from a
        # wedged core) usually clear on a fresh attempt.
        res = run_bass_kernel_spmd(nc, in_maps, core_ids=list(range(NUM_EXPERTS)))
    global LAST_RESULTS
    LAST_RESULTS = res

    out = np.zeros((T, D), dtype=np.float32)
    for e in range(NUM_EXPERTS):
        ids, g = tiers[e]
        n_dev = min(len(ids), SLOTS)
        r = res.results[e]["out"]
        out[ids[:n_dev]] += r[:n_dev]
        if len(ids) > SLOTS:
            # capacity overflow: exact fp32 FFN on host for the tail
            ho_ids = ids[SLOTS:]
            h = _gelu(xf[ho_ids] @ w1[e] + b1[e])
            out[ho_ids] += (h @ w2[e]) * g[SLOTS:][:, None]
    if b2.any():
        out += g0[:, None] * b2[sel0] + g1[:, None] * b2[sel1]
    return out.reshape(B, S, D)


# revision 34
# speedup vs baseline: 1.0173x; 1.0173x over previous
"""MoE layer (8 experts, top-2) Trainium2 kernel — fp16/fp8 tiers + capacity.

Expert-parallel (per sharding hint): host computes the tiny router
(logits -> top-2 -> softmax gates) and dispatches tokens to the 8 NeuronCores
by selected expert; core e runs expert e's FFN.  Each expert's token-pairs
are ranked by gate and split into fixed-size tiers (same sizes on every core
so one NEFF serves all 8 SPMD):

  f16 tier (N16=2048, largest gates)  fp16 mm1 + fp16 mm2           cost 1.00
  mid tier (NM=896)                   fp16 mm1 + fp8 DoubleRow mm2  cost 0.75
  fp8 tier (N8=512, smallest gates)   fp8 DR mm1 + fp8 DR mm2       cost 0.50
  beyond SLOTS=3456 (expert capacity) exact fp32 on host (small tail)

The mid tier quantizes only the mm2 inputs (h, w2), sitting slightly below
the f16<->fp8 cost/error^2 mixing chord.  h entering fp8 mm2 is centered
(h - 0.28125, its mean) and scaled x32 before e4m3 quantization — DVE does
the affine from an f16 staging tile, and the constant term is added back
exactly on the host — cutting the h quantization error ~19%.  Tier sizes
were tuned against a host-side ml_dtypes simulator that predicts the
measured HW rel-err to ~1e-4 (target ~0.0195 of the 2e-2 budget).

fp8 (e4m3) DoubleRow matmuls run 2x the fp16 PE rate.  All fp8 operands are
scaled into e4m3's normal range (x*16, w1*64, w2*128) to avoid subnormal
quantization loss; the scales are undone exactly via the activation's
input scale (1/1024, power of two) and host-side gate scaling (g/128).
fp16 (not bf16) for everything else makes the high tier's quantization
error negligible, freeing the 2e-2 rel-err budget for bigger fp8 tiers.
Expert overflow past SLOTS (capacity factor SLOTS/4096) is computed exactly
on the host (tens of GFLOP, sub-second) — standard MoE capacity handling,
with recompute instead of token dropping.

Execution order puts the full-fp8 tier FIRST: its PE work needs only ~9MB
of DMA (fp8 weights + tokens), hiding the cold-start weight load; w1q is
split into 8 per-slice tiles so the first matmul only waits on slice 0.
The mid tier runs second (streams w1h, needs only w2q for mm2); the f16
tier last (needs w2h, which loads during the earlier phases).

SBUF keeps w2h(f16) + w1q/w2q(fp8) resident; w1h streams per supertile.
All streamed tensors are pre-permuted on the host into SBUF consumption
order so each DMA moves >=4KB contiguous per partition.  fp8 x tiles alias
the f16 tiles' storage via bitcast.  All matmuls accumulate in fp32 PSUM.
"""

import numpy as np
import ml_dtypes

P = 128
D_MODEL = 1024
D_FF = 4096
NUM_EXPERTS = 8
KT1 = D_MODEL // P   # 8  k-tiles for mm1
KT2 = D_FF // P      # 32 k-tiles for mm2
FFC = D_FF // P      # 32 ff chunks (mm1 output partition tiles)
NH = D_MODEL // 2    # 512, free dim of mm2 matmuls
W1SL = 256           # ff columns per streamed f16-w1 slice
N_SL = D_FF // W1SL  # 16 slices per supertile
W1QSL = 512          # ff columns per fp8-w1 slice
N_QSL = D_FF // W1QSL

N16 = 2048           # f16 tier slots per expert
NM = 896             # mid tier (f16 mm1 + fp8 mm2)
N8 = 512             # full-fp8 tier
NQ = NM + N8         # rows whose mm2 goes through w2q (gates get /(S2*SH))
SLOTS = N16 + NM + N8  # 3456 = expert capacity; overflow -> exact host

SX = 16.0            # x scale into e4m3
S1 = 64.0            # w1 scale into e4m3
S2 = 128.0           # w2 scale into e4m3 (undone via gates)
ACT_SCALE = 1.0 / (SX * S1)
# h fed to fp8 mm2 is centered then scaled: h8 = e4m3(SH*(h - HC)).  Centering
# shrinks |h - HC| (h = gelu(z) has mean ~0.28), cutting the h quantization
# error ~19%; the constant term HC @ w2q is added back exactly on the host.
SH = 32.0
HC = 0.28125         # == 9/32, so SH*HC = 9 exactly

F16 = np.float16
F8 = ml_dtypes.float8_e4m3

_NC_CACHE: dict = {}
LAST_RESULTS = None  # BassKernelResults of the most recent device run

# Chunk sizes are chosen so weight loads hide behind the stream: fp8 DR
# matmuls are LDWEIGHTS-bound below 512-wide streams, f16 below 256-wide.
CH16 = [512, 512, 512, 512]        # sums to N16
CHM = [512, 384]                   # sums to NM
CH8 = [512]                        # sums to N8
assert sum(CH16) == N16 and sum(CHM) == NM and sum(CH8) == N8


def _build(act="Gelu"):
    import concourse.bacc as bacc
    import concourse.tile as tile
    import concourse.mybir as mybir

    f16 = mybir.dt.float16
    f8 = mybir.dt.float8e4
    f32 = mybir.dt.float32
    DR = mybir.MatmulPerfMode.DoubleRow
    GELU = getattr(mybir.ActivationFunctionType, act)

    nst16 = len(CH16)
    nstm = len(CHM)
    nst8 = len(CH8)

    nc = bacc.Bacc("TRN2", target_bir_lowering=False, debug=False)
    # All inputs are host-side pre-permuted into [partition, ...consumption
    # order...] so every DMA line is long and contiguous.  xT16 holds the
    # f16-tier supertiles then the mid tier's; xT8 the full-fp8 tier's.
    # Device out rows are in rank order: f16 | mid | fp8.
    xT16_d = nc.dram_tensor("xT16", [P, nst16 + nstm, KT1, 512], f16, kind="ExternalInput").ap()
    xT8_d = nc.dram_tensor("xT8", [P, nst8, KT1, 512], f8, kind="ExternalInput").ap()
    w1h_d = nc.dram_tensor("w1h", [P, N_SL, KT1, W1SL], f16, kind="ExternalInput").ap()
    w1q_d = nc.dram_tensor("w1q", [P, N_QSL, KT1, W1QSL], f8, kind="ExternalInput").ap()
    w2h_d = nc.dram_tensor("w2h", [P, KT2, D_MODEL], f16, kind="ExternalInput").ap()
    w2q_d = nc.dram_tensor("w2q", [P, KT2, D_MODEL], f8, kind="ExternalInput").ap()
    b1_d = nc.dram_tensor("b1", [P, FFC], f32, kind="ExternalInput").ap()
    g16_d = nc.dram_tensor("g16", [P, N16 // P], f32, kind="ExternalInput").ap()
    gq_d = nc.dram_tensor("gq", [P, NQ // P], f32, kind="ExternalInput").ap()
    out_d = nc.dram_tensor("out", [SLOTS, D_MODEL], f32, kind="ExternalOutput").ap()

    with tile.TileContext(nc) as tc:
        with (
            tc.tile_pool(name="wpool", bufs=1) as wpool,
            tc.tile_pool(name="w1pool", bufs=5) as w1pool,
            tc.tile_pool(name="xpool", bufs=2) as xpool,
            tc.tile_pool(name="hpool", bufs=1) as hpool,
            tc.tile_pool(name="hsp", bufs=2) as hsp,
            tc.tile_pool(name="opool", bufs=3) as opool,
            tc.tile_pool(name="php", bufs=4, space="PSUM") as php,
            tc.tile_pool(name="pyp", bufs=4, space="PSUM") as pyp,
        ):
            # --- prologue: cold-start DMA is per-transfer-latency-bound, so
            # the first matmul's dependencies (x8 supertile + w1q slice 0) are
            # few fat transfers issued ahead of everything else.  w1q lives in
            # 8 per-slice tiles so matmuls unlock slice by slice.
            xt8s = []
            for st in range(nst8):
                xtA = xpool.tile([P, KT1, 512], f16, tag="xt")
                xt8s.append(xtA[:].bitcast(f8))
                nc.sync.dma_start(xt8s[st][:, :, :512], xT8_d[:, st])
            w1q_sl = []
            for sl in range(N_QSL):
                t = wpool.tile([P, KT1, W1QSL], f8, tag=f"w1q{sl}")
                nc.sync.dma_start(t[:], w1q_d[:, sl])
                w1q_sl.append(t)
                if sl == 0:
                    # b1 gates the first activation (and through PSUM reuse,
                    # the 5th ffc's matmuls) — land it right after slice 0.
                    b1_sb = wpool.tile([P, FFC], f32, tag="b1")
                    nc.sync.dma_start(b1_sb[:], b1_d[:])
            gq_sb = wpool.tile([P, NQ // P], f32, tag="gq")
            nc.sync.dma_start(gq_sb[:], gq_d[:])
            g16_sb = wpool.tile([P, N16 // P], f32, tag="g16")
            nc.sync.dma_start(g16_sb[:], g16_d[:])
            w2q_sb = wpool.tile([P, KT2, D_MODEL], f8, tag="w2q")
            for q in range(4):
                nc.sync.dma_start(
                    w2q_sb[:, q * 8 : (q + 1) * 8, :], w2q_d[:, q * 8 : (q + 1) * 8, :]
                )
            def center_h(dst_view, ph, tok_len, ffc, act_scale):
                """GELU to f16 staging, then DVE affine SH*(h-HC) into fp8."""
                hs = hsp.tile([P, 512], f16, tag="hs")
                nc.scalar.activation(
                    hs[:, :tok_len], ph[:, :tok_len], GELU,
                    bias=b1_sb[:, ffc : ffc + 1], scale=act_scale,
                )
                nc.vector.tensor_scalar(
                    dst_view, hs[:, :tok_len], SH, -SH * HC,
                    op0=mybir.AluOpType.mult, op1=mybir.AluOpType.add,
                )

            def mm1_fp8(xt8, ht_ffc_view, tok_len):
                """DR mm1 on scaled fp8; activation undoes the x/w1 scales."""
                for ffc in range(FFC):
                    ph = php.tile([P, 512], f32, tag="ph")
                    sl, col = divmod(ffc * P, W1QSL)
                    for k2 in range(KT1 // 2):
                        nc.tensor.matmul(
                            ph[:, :tok_len],
                            w1q_sl[sl][:, 2 * k2 : 2 * k2 + 2, col : col + P],
                            xt8[:, 2 * k2 : 2 * k2 + 2, :tok_len],
                            start=(k2 == 0),
                            stop=(k2 == KT1 // 2 - 1),
                            perf_mode=DR,
                        )
                    center_h(ht_ffc_view(ffc), ph, tok_len, ffc, ACT_SCALE)

            def mm1_f16(xt, w1sl, ht_ffc_view, tok_len, center=False):
                for ffc in range(FFC):
                    ph = php.tile([P, 512], f32, tag="ph")
                    sl, col = divmod(ffc * P, W1SL)
                    for kt in range(KT1):
                        nc.tensor.matmul(
                            ph[:, :tok_len],
                            w1sl[sl][:, kt, col : col + P],
                            xt[:, kt, :tok_len],
                            start=(kt == 0),
                            stop=(kt == KT1 - 1),
                        )
                    if center:
                        center_h(ht_ffc_view(ffc), ph, tok_len, ffc, 1.0)
                    else:
                        nc.scalar.activation(
                            ht_ffc_view(ffc), ph[:, :tok_len], GELU,
                            bias=b1_sb[:, ffc : ffc + 1], scale=1.0,
                        )

            def mm2_fp8(ht8, col0, tok_len, out_row0, gcol0, drain_split=False):
                """DR mm2 through scaled w2q; gates (pre-divided by S2) undo it."""
                n_mt = tok_len // P
                for mt in range(n_mt):
                    if drain_split and mt == n_mt - 1:
                        for nb in (0, 1):
                            py = pyp.tile([P, NH], f32, tag="py")
                            for k2 in range(KT2 // 2):
                                nc.tensor.matmul(
                                    py,
                                    ht8[:, 2 * k2 : 2 * k2 + 2, col0 + mt * P : col0 + (mt + 1) * P],
                                    w2q_sb[:, 2 * k2 : 2 * k2 + 2, nb * NH : (nb + 1) * NH],
                                    start=(k2 == 0), stop=(k2 == KT2 // 2 - 1),
                                    perf_mode=DR,
                                )
                            ot = opool.tile([P, NH], f32, tag="ot")
                            nc.vector.tensor_scalar_mul(
                                ot[:], py[:], gq_sb[:, gcol0 + mt : gcol0 + mt + 1]
                            )
                            nc.sync.dma_start(
                                out_d[
                                    out_row0 + mt * P : out_row0 + (mt + 1) * P,
                                    nb * NH : (nb + 1) * NH,
                                ],
                                ot[:],
                            )
                        continue
                    py0 = pyp.tile([P, NH], f32, tag="py")
                    py1 = pyp.tile([P, NH], f32, tag="py")
                    for k2 in range(KT2 // 2):
                        lhsT = ht8[:, 2 * k2 : 2 * k2 + 2, col0 + mt * P : col0 + (mt + 1) * P]
                        nc.tensor.matmul(
                            py0, lhsT, w2q_sb[:, 2 * k2 : 2 * k2 + 2, 0:NH],
                            start=(k2 == 0), stop=(k2 == KT2 // 2 - 1), perf_mode=DR,
                        )
                        nc.tensor.matmul(
                            py1, lhsT, w2q_sb[:, 2 * k2 : 2 * k2 + 2, NH:D_MODEL],
                            start=(k2 == 0), stop=(k2 == KT2 // 2 - 1), perf_mode=DR,
                        )
                    for nb, py in ((0, py0), (1, py1)):
                        ot = opool.tile([P, NH], f32, tag="ot")
                        nc.vector.tensor_scalar_mul(
                            ot[:], py[:], gq_sb[:, gcol0 + mt : gcol0 + mt + 1]
                        )
                        nc.sync.dma_start(
                            out_d[
                                out_row0 + mt * P : out_row0 + (mt + 1) * P,
                                nb * NH : (nb + 1) * NH,
                            ],
                            ot[:],
                        )

            def mm2_f16(ht, tok_len, out_row0, gcol0, drain_split=False):
                n_mt = tok_len // P
                for mt in range(n_mt):
                    if drain_split and mt == n_mt - 1:
                        # Kernel-final block: run four quarter-width output
                        # pieces sequentially so each piece's gate-mult +
                        # store overlaps the next piece's matmuls; only the
                        # last 256-col chain trails the final matmul.
                        NQW = D_MODEL // 4
                        for nb in range(4):
                            py = pyp.tile([P, NH], f32, tag="py")
                            for kt in range(KT2):
                                nc.tensor.matmul(
                                    py[:, :NQW], ht[:, kt, mt * P : (mt + 1) * P],
                                    w2h_sb[:, kt, nb * NQW : (nb + 1) * NQW],
                                    start=(kt == 0), stop=(kt == KT2 - 1),
                                )
                            ot = opool.tile([P, NH], f32, tag="ot")
                            nc.vector.tensor_scalar_mul(
                                ot[:, :NQW], py[:, :NQW],
                                g16_sb[:, gcol0 + mt : gcol0 + mt + 1],
                            )
                            nc.sync.dma_start(
                                out_d[
                                    out_row0 + mt * P : out_row0 + (mt + 1) * P,
                                    nb * NQW : (nb + 1) * NQW,
                                ],
                                ot[:, :NQW],
                            )
                        continue
                    py0 = pyp.tile([P, NH], f32, tag="py")
                    py1 = pyp.tile([P, NH], f32, tag="py")
                    for kt in range(KT2):
                        lhsT = ht[:, kt, mt * P : (mt + 1) * P]
                        nc.tensor.matmul(
                            py0, lhsT, w2h_sb[:, kt, 0:NH],
                            start=(kt == 0), stop=(kt == KT2 - 1),
                        )
                        nc.tensor.matmul(
                            py1, lhsT, w2h_sb[:, kt, NH:D_MODEL],
                            start=(kt == 0), stop=(kt == KT2 - 1),
                        )
                    for nb, py in ((0, py0), (1, py1)):
                        ot = opool.tile([P, NH], f32, tag="ot")
                        nc.vector.tensor_scalar_mul(
                            ot[:], py[:], g16_sb[:, gcol0 + mt : gcol0 + mt + 1]
                        )
                        nc.sync.dma_start(
                            out_d[
                                out_row0 + mt * P : out_row0 + (mt + 1) * P,
                                nb * NH : (nb + 1) * NH,
                            ],
                            ot[:],
                        )

            # ---------- full-fp8 tier (DMA-cheap warmup) ----------
            # All supertiles' mm1 run back-to-back into disjoint column
            # ranges of one h tile; their mm2 follow.
            ht_f8 = hpool.tile([P, KT2, 512], f16, tag="ht")
            ht8 = ht_f8[:].bitcast(f8)  # [P, KT2, 1024] view
            c8 = [0]
            for L in CH8:
                c8.append(c8[-1] + L)
            for st in range(nst8):
                mm1_fp8(
                    xt8s[st],
                    lambda ffc, st=st: ht8[:, ffc, c8[st] : c8[st + 1]],
                    CH8[st],
                )

            # mid/f16-tier streams launch behind the fp8 weights; the fp8 mm2
            # below only needs w2q, which is already queued.
            xt0 = xpool.tile([P, KT1, 512], f16, tag="xt")
            nc.sync.dma_start(xt0[:], xT16_d[:, nst16])  # mid supertile 0
            w1sl0 = []
            for kt in range(2):
                t = w1pool.tile([P, KT1, W1SL], f16, tag="w1sl")
                nc.sync.dma_start(t[:], w1h_d[:, kt])
                w1sl0.append(t)
            w2h_sb = wpool.tile([P, KT2, D_MODEL], f16, tag="w2h")
            for q in range(4):
                nc.sync.dma_start(
                    w2h_sb[:, q * 8 : (q + 1) * 8, :], w2h_d[:, q * 8 : (q + 1) * 8, :]
                )

            for st in range(nst8):
                mm2_fp8(
                    ht8, c8[st], CH8[st],
                    N16 + NM + c8[st], NM // P + c8[st] // P,
                )

            # ---------- mid tier: f16 mm1 -> fp8 h -> DR mm2 ----------
            tokm = 0
            for stm, tok_len in enumerate(CHM):
                if stm == 0:
                    xt = xt0
                    w1sl = list(w1sl0)
                else:
                    xt = xpool.tile([P, KT1, 512], f16, tag="xt")
                    nc.sync.dma_start(xt[:], xT16_d[:, nst16 + stm])
                    w1sl = []
                for sl in range(len(w1sl), N_SL):
                    t = w1pool.tile([P, KT1, W1SL], f16, tag="w1sl")
                    nc.sync.dma_start(t[:], w1h_d[:, sl])
                    w1sl.append(t)
                htm = hpool.tile([P, KT2, 512], f16, tag="ht")
                htm8 = htm[:].bitcast(f8)  # [P, KT2, 1024] view
                col0 = 512 * (stm % 2)
                mm1_f16(
                    xt, w1sl,
                    lambda ffc: htm8[:, ffc, col0 : col0 + tok_len],
                    tok_len, center=True,
                )
                mm2_fp8(htm8, col0, tok_len, N16 + tokm, tokm // P)
                tokm += tok_len

            # ---------- f16 tier ----------
            tok0 = 0
            for st, tok_len in enumerate(CH16):
                xt = xpool.tile([P, KT1, 512], f16, tag="xt")
                nc.sync.dma_start(xt[:], xT16_d[:, st])
                w1sl = []
                for sl in range(N_SL):
                    t = w1pool.tile([P, KT1, W1SL], f16, tag="w1sl")
                    nc.sync.dma_start(t[:], w1h_d[:, sl])
                    w1sl.append(t)
                ht = hpool.tile([P, KT2, 512], f16, tag="ht")
                mm1_f16(xt, w1sl, lambda ffc: ht[:, ffc, :tok_len], tok_len)
                mm2_f16(ht, tok_len, tok0, tok0 // P, drain_split=(st == nst16 - 1))
                tok0 += tok_len
    nc.compile()
    return nc


def _get_nc(act="Gelu"):
    if act not in _NC_CACHE:
        _NC_CACHE[act] = _build(act)
    return _NC_CACHE[act]


def _perm_w(w, kt, cols):
    """[kt*P, cols] -> [P, kt, cols] with row = kt*P + p."""
    return np.ascontiguousarray(w.reshape(kt, P, cols).transpose(1, 0, 2))


def _perm_w_sliced(w, kt, n_sl, slw):
    """[kt*P, n_sl*slw] -> [P, n_sl, kt, slw]."""
    return np.ascontiguousarray(w.reshape(kt, P, n_sl, slw).transpose(1, 2, 0, 3))


def _pack_x(xcols, chunks, dtype):
    """[D_MODEL, n] columns -> [P, n_st, KT1, 512] supertile blocks."""
    out = np.zeros((P, len(chunks), KT1, 512), dtype=dtype)
    t0 = 0
    for st, L in enumerate(chunks):
        blk = xcols[:, t0 : t0 + L].reshape(KT1, P, L).transpose(1, 0, 2)
        out[:, st, :, :L] = blk
        t0 += L
    return out


def _erf(v):
    try:
        from scipy.special import erf
        return erf(v)
    except ImportError:
        # Abramowitz & Stegun 7.1.26 (|err| < 1.5e-7), numpy-only fallback.
        s = np.sign(v)
        a = np.abs(v)
        t = 1.0 / (1.0 + 0.3275911 * a)
        y = 1.0 - (
            ((((1.061405429 * t - 1.453152027) * t) + 1.421413741) * t
             - 0.284496736) * t + 0.254829592
        ) * t * np.exp(-a * a)
        return s * y


def _gelu(v):
    return v * 0.5 * (1.0 + _erf(v / np.sqrt(2.0)))


def kernel(x, router_w, router_b, w1, b1, w2, b2):
    from concourse.bass_utils import run_bass_kernel_spmd

    x = np.asarray(x, dtype=np.float32)
    router_w = np.asarray(router_w, dtype=np.float32)
    router_b = np.asarray(router_b, dtype=np.float32)
    w1 = np.asarray(w1, dtype=np.float32)
    b1 = np.asarray(b1, dtype=np.float32)
    w2 = np.asarray(w2, dtype=np.float32)
    b2 = np.asarray(b2, dtype=np.float32)

    B, S, D = x.shape
    T = B * S
    xf = x.reshape(T, D)

    # --- host router: top-2 + softmax gates (tiny: T x D x 8) ---
    logits = xf @ router_w + router_b                      # [T, 8] fp32
    sel0 = np.argmax(logits, axis=1)
    l0 = logits[np.arange(T), sel0]
    masked = logits.copy()
    masked[np.arange(T), sel0] = -np.inf
    sel1 = np.argmax(masked, axis=1)
    l1 = masked[np.arange(T), sel1]
    e1 = np.exp(l1 - l0)
    g0 = 1.0 / (1.0 + e1)
    g1 = e1 / (1.0 + e1)

    # --- dispatch: per expert, rank pairs by gate; tier by rank.
    # Device slots cover ranks [0, SLOTS) in order f16 | mid | fp8; ranks
    # beyond SLOTS (capacity overflow) are computed exactly on the host.
    tiers = []  # per expert: (ids, g) rank-sorted
    for e in range(NUM_EXPERTS):
        ids0 = np.nonzero(sel0 == e)[0]
        ids1 = np.nonzero(sel1 == e)[0]
        ids = np.concatenate([ids0, ids1])
        g = np.concatenate([g0[ids0], g1[ids1]]).astype(np.float32)
        order = np.argsort(-g, kind="stable")
        tiers.append((ids[order], g[order]))

    nc = _get_nc()

    in_maps = []
    for e in range(NUM_EXPERTS):
        ids, g = tiers[e]
        ids_dev, g_dev = ids[:SLOTS], g[:SLOTS]
        n_dev = len(ids_dev)
        x16 = np.zeros((D_MODEL, N16 + NM), dtype=np.float32)
        x16[:, : min(n_dev, N16 + NM)] = xf[ids_dev[: N16 + NM]].T
        x8 = np.zeros((D_MODEL, N8), dtype=np.float32)
        if n_dev > N16 + NM:
            x8[:, : n_dev - (N16 + NM)] = xf[ids_dev[N16 + NM :]].T
        gp = np.zeros((SLOTS,), dtype=np.float32)
        gp[:n_dev] = g_dev
        # rows whose mm2 goes through w2q: psum = (SH*(h-HC)) @ (S2*w2)
        gp[N16:] /= S2 * SH
        in_maps.append(
            {
                "xT16": _pack_x(x16.astype(F16), CH16 + CHM, F16),
                "xT8": _pack_x(
                    np.clip(x8 * SX, -240.0, 240.0).astype(F8), CH8, F8
                ),
                "w1h": _perm_w_sliced(w1[e].astype(F16), KT1, N_SL, W1SL),
                "w1q": _perm_w_sliced((w1[e] * S1).astype(F8), KT1, N_QSL, W1QSL),
                "w2h": _perm_w(w2[e].astype(F16), KT2, D_MODEL),
                "w2q": _perm_w((w2[e] * S2).astype(F8), KT2, D_MODEL),
                "b1": np.ascontiguousarray(b1[e].reshape(FFC, P).T),
                "g16": np.ascontiguousarray(gp[:N16].reshape(N16 // P, P).T),
                "gq": np.ascontiguousarray(gp[N16:].reshape(NQ // P, P).T),
            }
        )

    try:
        res = run_bass_kernel_spmd(nc, in_maps, core_ids=list(range(NUM_EXPERTS)))
    except Exception:
        # Transient device errors (e.g. NRT_EXEC_UNIT_UNRECOVERABLE from a
        # wedged core) usually clear on a fresh attempt.
        res = run_bass_kernel_spmd(nc, in_maps, core_ids=list(range(NUM_EXPERTS)))
    global LAST_RESULTS
    LAST_RESULTS = res

    out = np.zeros((T, D), dtype=np.float32)
    for e in range(NUM_EXPERTS):
        ids, g = tiers[e]
        n_dev = min(len(ids), SLOTS)
        r = res.results[e]["out"]
        out[ids[:n_dev]] += r[:n_dev]
        # add back the centering constant: h@w2q = (h-HC)@w2q + HC*colsum(w2q)
        w2q_deq = (w2[e] * S2).astype(F8).astype(np.float32) / S2
        corr = HC * w2q_deq.sum(axis=0)
        out[ids[N16:n_dev]] += np.outer(g[N16:n_dev], corr)
        if len(ids) > SLOTS:
            # capacity overflow: exact fp32 FFN on host for the tail
            ho_ids = ids[SLOTS:]
            h = _gelu(xf[ho_ids] @ w1[e] + b1[e])
            out[ho_ids] += (h @ w2[e]) * g[SLOTS:][:, None]
    if b2.any():
        out += g0[:, None] * b2[sel0] + g1[:, None] * b2[sel1]
    return out.reshape(B, S, D)


# revision 38
# speedup vs baseline: 1.0175x; 1.0002x over previous
"""MoE layer (8 experts, top-2) Trainium2 kernel — fp16/fp8 tiers + capacity.

Expert-parallel (per sharding hint): host computes the tiny router
(logits -> top-2 -> softmax gates) and dispatches tokens to the 8 NeuronCores
by selected expert; core e runs expert e's FFN.  Each expert's token-pairs
are ranked by gate and split into fixed-size tiers (same sizes on every core
so one NEFF serves all 8 SPMD):

  f16 tier (N16=2048, largest gates)  fp16 mm1 + fp16 mm2           cost 1.00
  mid tier (NM=896)                   fp16 mm1 + fp8 DoubleRow mm2  cost 0.75
  fp8 tier (N8=512, smallest gates)   fp8 DR mm1 + fp8 DR mm2       cost 0.50
  beyond SLOTS=3456 (expert capacity) exact fp32 on host (small tail)

The mid tier quantizes only the mm2 inputs (h, w2), sitting slightly below
the f16<->fp8 cost/error^2 mixing chord.  h entering fp8 mm2 is centered
(h - 0.28125, its mean) and scaled x32 before e4m3 quantization — DVE does
the affine from an f16 staging tile, and the constant term is added back
exactly on the host — cutting the h quantization error ~19%.  Tier sizes
were tuned against a host-side ml_dtypes simulator that predicts the
measured HW rel-err to ~1e-4 (target ~0.0195 of the 2e-2 budget).

fp8 (e4m3) DoubleRow matmuls run 2x the fp16 PE rate.  All fp8 operands are
scaled into e4m3's normal range (x*16, w1*64, w2*128) to avoid subnormal
quantization loss; the scales are undone exactly via the activation's
input scale (1/1024, power of two) and host-side gate scaling (g/128).
fp16 (not bf16) for everything else makes the high tier's quantization
error negligible, freeing the 2e-2 rel-err budget for bigger fp8 tiers.
Expert overflow past SLOTS (capacity factor SLOTS/4096) is computed exactly
on the host (tens of GFLOP, sub-second) — standard MoE capacity handling,
with recompute instead of token dropping.

Execution order puts the full-fp8 tier FIRST: its PE work needs only ~9MB
of DMA (fp8 weights + tokens), hiding the cold-start weight load; w1q is
split into 8 per-slice tiles so the first matmul only waits on slice 0.
The mid tier runs second (streams w1h, needs only w2q for mm2); the f16
tier last (needs w2h, which loads during the earlier phases).

SBUF keeps w2h(f16) + w1q/w2q(fp8) resident; w1h streams per supertile.
All streamed tensors are pre-permuted on the host into SBUF consumption
order so each DMA moves >=4KB contiguous per partition.  fp8 x tiles alias
the f16 tiles' storage via bitcast.  All matmuls accumulate in fp32 PSUM.
"""

import numpy as np
import ml_dtypes

P = 128
D_MODEL = 1024
D_FF = 4096
NUM_EXPERTS = 8
KT1 = D_MODEL // P   # 8  k-tiles for mm1
KT2 = D_FF // P      # 32 k-tiles for mm2
FFC = D_FF // P      # 32 ff chunks (mm1 output partition tiles)
NH = D_MODEL // 2    # 512, free dim of mm2 matmuls
W1SL = 256           # ff columns per streamed f16-w1 slice
N_SL = D_FF // W1SL  # 16 slices per supertile
W1QSL = 512          # ff columns per fp8-w1 slice
N_QSL = D_FF // W1QSL

N16 = 2048           # f16 tier slots per expert
NM = 896             # mid tier (f16 mm1 + fp8 mm2)
N8 = 512             # full-fp8 tier
NQ = NM + N8         # rows whose mm2 goes through w2q (gates get /(S2*SH))
SLOTS = N16 + NM + N8  # 3456 = expert capacity; overflow -> exact host

SX = 16.0            # x scale into e4m3
S1 = 64.0            # w1 scale into e4m3
S2 = 128.0           # w2 scale into e4m3 (undone via gates)
ACT_SCALE = 1.0 / (SX * S1)
# h fed to fp8 mm2 is centered then scaled: h8 = e4m3(SH*(h - HC)).  Centering
# shrinks |h - HC| (h = gelu(z) has mean ~0.28), cutting the h quantization
# error ~19%; the constant term HC @ w2q is added back exactly on the host.
SH = 32.0
HC = 0.28125         # == 9/32, so SH*HC = 9 exactly

F16 = np.float16
F8 = ml_dtypes.float8_e4m3

_NC_CACHE: dict = {}
LAST_RESULTS = None  # BassKernelResults of the most recent device run

# Chunk sizes are chosen so weight loads hide behind the stream: fp8 DR
# matmuls are LDWEIGHTS-bound below 512-wide streams, f16 below 256-wide.
CH16 = [512, 512, 512, 512]        # sums to N16
CHM = [512, 384]                   # sums to NM
CH8 = [512]                        # sums to N8
assert sum(CH16) == N16 and sum(CHM) == NM and sum(CH8) == N8


def _build(act="Gelu"):
    import concourse.bacc as bacc
    import concourse.tile as tile
    import concourse.mybir as mybir

    f16 = mybir.dt.float16
    f8 = mybir.dt.float8e4
    f32 = mybir.dt.float32
    DR = mybir.MatmulPerfMode.DoubleRow
    GELU = getattr(mybir.ActivationFunctionType, act)

    nst16 = len(CH16)
    nstm = len(CHM)
    nst8 = len(CH8)

    nc = bacc.Bacc("TRN2", target_bir_lowering=False, debug=False)
    # All inputs are host-side pre-permuted into [partition, ...consumption
    # order...] so every DMA line is long and contiguous.  xT16 holds the
    # f16-tier supertiles then the mid tier's; xT8 the full-fp8 tier's.
    # Device out rows are in rank order: f16 | mid | fp8.
    xT16_d = nc.dram_tensor("xT16", [P, nst16 + nstm, KT1, 512], f16, kind="ExternalInput").ap()
    xT8_d = nc.dram_tensor("xT8", [P, nst8, KT1, 512], f8, kind="ExternalInput").ap()
    w1h_d = nc.dram_tensor("w1h", [P, N_SL, KT1, W1SL], f16, kind="ExternalInput").ap()
    w1q_d = nc.dram_tensor("w1q", [P, N_QSL, KT1, W1QSL], f8, kind="ExternalInput").ap()
    w2h_d = nc.dram_tensor("w2h", [P, KT2, D_MODEL], f16, kind="ExternalInput").ap()
    w2q_d = nc.dram_tensor("w2q", [P, KT2, D_MODEL], f8, kind="ExternalInput").ap()
    b1_d = nc.dram_tensor("b1", [P, FFC], f32, kind="ExternalInput").ap()
    g16_d = nc.dram_tensor("g16", [P, N16 // P], f32, kind="ExternalInput").ap()
    gq_d = nc.dram_tensor("gq", [P, NQ // P], f32, kind="ExternalInput").ap()
    out_d = nc.dram_tensor("out", [SLOTS, D_MODEL], f16, kind="ExternalOutput").ap()

    with tile.TileContext(nc) as tc:
        with (
            tc.tile_pool(name="wpool", bufs=1) as wpool,
            tc.tile_pool(name="w1pool", bufs=5) as w1pool,
            tc.tile_pool(name="xpool", bufs=2) as xpool,
            tc.tile_pool(name="hpool", bufs=1) as hpool,
            tc.tile_pool(name="hsp", bufs=2) as hsp,
            tc.tile_pool(name="opool", bufs=3) as opool,
            tc.tile_pool(name="php", bufs=4, space="PSUM") as php,
            tc.tile_pool(name="pyp", bufs=4, space="PSUM") as pyp,
        ):
            # --- prologue: cold-start DMA is per-transfer-latency-bound, so
            # the first matmul's dependencies (x8 supertile + w1q slice 0) are
            # few fat transfers issued ahead of everything else.  w1q lives in
            # 8 per-slice tiles so matmuls unlock slice by slice.
            # The first matmul needs only x k-tiles 0-3 and w1q slice-0 cols
            # 0-255; splitting those transfers halves the cold-DMA bytes the
            # PE waits on (Tile tracks sub-tile write regions).
            xt8s = []
            for st in range(nst8):
                xtA = xpool.tile([P, KT1, 512], f16, tag="xt")
                xt8s.append(xtA[:].bitcast(f8))
                if st == 0:
                    nc.sync.dma_start(
                        xt8s[st][:, : KT1 // 2, :512], xT8_d[:, st, : KT1 // 2]
                    )
                else:
                    nc.sync.dma_start(xt8s[st][:, :, :512], xT8_d[:, st])
            w1q_sl = []
            for sl in range(N_QSL):
                t = wpool.tile([P, KT1, W1QSL], f8, tag=f"w1q{sl}")
                if sl == 0:
                    nc.sync.dma_start(t[:, :, :256], w1q_d[:, 0, :, :256])
                    nc.sync.dma_start(
                        xt8s[0][:, KT1 // 2 :, :512], xT8_d[:, 0, KT1 // 2 :]
                    )
                    nc.sync.dma_start(t[:, :, 256:], w1q_d[:, 0, :, 256:])
                    # b1 gates the first activation (and through PSUM reuse,
                    # the 5th ffc's matmuls) — land it right after slice 0.
                    b1_sb = wpool.tile([P, FFC], f32, tag="b1")
                    nc.sync.dma_start(b1_sb[:], b1_d[:])
                else:
                    nc.sync.dma_start(t[:], w1q_d[:, sl])
                w1q_sl.append(t)
            gq_sb = wpool.tile([P, NQ // P], f32, tag="gq")
            nc.sync.dma_start(gq_sb[:], gq_d[:])
            g16_sb = wpool.tile([P, N16 // P], f32, tag="g16")
            nc.sync.dma_start(g16_sb[:], g16_d[:])
            w2q_sb = wpool.tile([P, KT2, D_MODEL], f8, tag="w2q")
            for q in range(4):
                nc.sync.dma_start(
                    w2q_sb[:, q * 8 : (q + 1) * 8, :], w2q_d[:, q * 8 : (q + 1) * 8, :]
                )
            def center_h(dst_view, ph, tok_len, ffc, act_scale):
                """GELU to f16 staging, then DVE affine SH*(h-HC) into fp8."""
                hs = hsp.tile([P, 512], f16, tag="hs")
                nc.scalar.activation(
                    hs[:, :tok_len], ph[:, :tok_len], GELU,
                    bias=b1_sb[:, ffc : ffc + 1], scale=act_scale,
                )
                nc.vector.tensor_scalar(
                    dst_view, hs[:, :tok_len], SH, -SH * HC,
                    op0=mybir.AluOpType.mult, op1=mybir.AluOpType.add,
                )

            def mm1_fp8(xt8, ht_ffc_view, tok_len):
                """DR mm1 on scaled fp8; activation undoes the x/w1 scales."""
                for ffc in range(FFC):
                    ph = php.tile([P, 512], f32, tag="ph")
                    sl, col = divmod(ffc * P, W1QSL)
                    for k2 in range(KT1 // 2):
                        nc.tensor.matmul(
                            ph[:, :tok_len],
                            w1q_sl[sl][:, 2 * k2 : 2 * k2 + 2, col : col + P],
                            xt8[:, 2 * k2 : 2 * k2 + 2, :tok_len],
                            start=(k2 == 0),
                            stop=(k2 == KT1 // 2 - 1),
                            perf_mode=DR,
                        )
                    center_h(ht_ffc_view(ffc), ph, tok_len, ffc, ACT_SCALE)

            def mm1_f16(xt, w1sl, ht_ffc_view, tok_len, center=False):
                for ffc in range(FFC):
                    ph = php.tile([P, 512], f32, tag="ph")
                    sl, col = divmod(ffc * P, W1SL)
                    for kt in range(KT1):
                        nc.tensor.matmul(
                            ph[:, :tok_len],
                            w1sl[sl][:, kt, col : col + P],
                            xt[:, kt, :tok_len],
                            start=(kt == 0),
                            stop=(kt == KT1 - 1),
                        )
                    if center:
                        center_h(ht_ffc_view(ffc), ph, tok_len, ffc, 1.0)
                    else:
                        nc.scalar.activation(
                            ht_ffc_view(ffc), ph[:, :tok_len], GELU,
                            bias=b1_sb[:, ffc : ffc + 1], scale=1.0,
                        )

            def mm2_fp8(ht8, col0, tok_len, out_row0, gcol0, drain_split=False):
                """DR mm2 through scaled w2q; gates (pre-divided by S2) undo it."""
                n_mt = tok_len // P
                for mt in range(n_mt):
                    if drain_split and mt == n_mt - 1:
                        for nb in (0, 1):
                            py = pyp.tile([P, NH], f32, tag="py")
                            for k2 in range(KT2 // 2):
                                nc.tensor.matmul(
                                    py,
                                    ht8[:, 2 * k2 : 2 * k2 + 2, col0 + mt * P : col0 + (mt + 1) * P],
                                    w2q_sb[:, 2 * k2 : 2 * k2 + 2, nb * NH : (nb + 1) * NH],
                                    start=(k2 == 0), stop=(k2 == KT2 // 2 - 1),
                                    perf_mode=DR,
                                )
                            ot = opool.tile([P, NH], f16, tag="ot")
                            nc.vector.tensor_scalar_mul(
                                ot[:], py[:], gq_sb[:, gcol0 + mt : gcol0 + mt + 1]
                            )
                            nc.sync.dma_start(
                                out_d[
                                    out_row0 + mt * P : out_row0 + (mt + 1) * P,
                                    nb * NH : (nb + 1) * NH,
                                ],
                                ot[:],
                            )
                        continue
                    py0 = pyp.tile([P, NH], f32, tag="py")
                    py1 = pyp.tile([P, NH], f32, tag="py")
                    for k2 in range(KT2 // 2):
                        lhsT = ht8[:, 2 * k2 : 2 * k2 + 2, col0 + mt * P : col0 + (mt + 1) * P]
                        nc.tensor.matmul(
                            py0, lhsT, w2q_sb[:, 2 * k2 : 2 * k2 + 2, 0:NH],
                            start=(k2 == 0), stop=(k2 == KT2 // 2 - 1), perf_mode=DR,
                        )
                        nc.tensor.matmul(
                            py1, lhsT, w2q_sb[:, 2 * k2 : 2 * k2 + 2, NH:D_MODEL],
                            start=(k2 == 0), stop=(k2 == KT2 // 2 - 1), perf_mode=DR,
                        )
                    for nb, py in ((0, py0), (1, py1)):
                        ot = opool.tile([P, NH], f16, tag="ot")
                        nc.vector.tensor_scalar_mul(
                            ot[:], py[:], gq_sb[:, gcol0 + mt : gcol0 + mt + 1]
                        )
                        nc.sync.dma_start(
                            out_d[
                                out_row0 + mt * P : out_row0 + (mt + 1) * P,
                                nb * NH : (nb + 1) * NH,
                            ],
                            ot[:],
                        )

            def mm2_f16(ht, tok_len, out_row0, gcol0, drain_split=False):
                n_mt = tok_len // P
                for mt in range(n_mt):
                    if drain_split and mt == n_mt - 1:
                        # Kernel-final block: run four quarter-width output
                        # pieces sequentially so each piece's gate-mult +
                        # store overlaps the next piece's matmuls; only the
                        # last 256-col chain trails the final matmul.
                        NQW = D_MODEL // 4
                        for nb in range(4):
                            py = pyp.tile([P, NH], f32, tag="py")
                            for kt in range(KT2):
                                nc.tensor.matmul(
                                    py[:, :NQW], ht[:, kt, mt * P : (mt + 1) * P],
                                    w2h_sb[:, kt, nb * NQW : (nb + 1) * NQW],
                                    start=(kt == 0), stop=(kt == KT2 - 1),
                                )
                            ot = opool.tile([P, NH], f16, tag="ot")
                            nc.vector.tensor_scalar_mul(
                                ot[:, :NQW], py[:, :NQW],
                                g16_sb[:, gcol0 + mt : gcol0 + mt + 1],
                            )
                            nc.sync.dma_start(
                                out_d[
                                    out_row0 + mt * P : out_row0 + (mt + 1) * P,
                                    nb * NQW : (nb + 1) * NQW,
                                ],
                                ot[:, :NQW],
                            )
                        continue
                    py0 = pyp.tile([P, NH], f32, tag="py")
                    py1 = pyp.tile([P, NH], f32, tag="py")
                    for kt in range(KT2):
                        lhsT = ht[:, kt, mt * P : (mt + 1) * P]
                        nc.tensor.matmul(
                            py0, lhsT, w2h_sb[:, kt, 0:NH],
                            start=(kt == 0), stop=(kt == KT2 - 1),
                        )
                        nc.tensor.matmul(
                            py1, lhsT, w2h_sb[:, kt, NH:D_MODEL],
                            start=(kt == 0), stop=(kt == KT2 - 1),
                        )
                    for nb, py in ((0, py0), (1, py1)):
                        ot = opool.tile([P, NH], f16, tag="ot")
                        nc.vector.tensor_scalar_mul(
                            ot[:], py[:], g16_sb[:, gcol0 + mt : gcol0 + mt + 1]
                        )
                        nc.sync.dma_start(
                            out_d[
                                out_row0 + mt * P : out_row0 + (mt + 1) * P,
                                nb * NH : (nb + 1) * NH,
                            ],
                            ot[:],
                        )

            # ---------- full-fp8 tier (DMA-cheap warmup) ----------
            # All supertiles' mm1 run back-to-back into disjoint column
            # ranges of one h tile; their mm2 follow.
            ht_f8 = hpool.tile([P, KT2, 512], f16, tag="ht")
            ht8 = ht_f8[:].bitcast(f8)  # [P, KT2, 1024] view
            c8 = [0]
            for L in CH8:
                c8.append(c8[-1] + L)
            for st in range(nst8):
                mm1_fp8(
                    xt8s[st],
                    lambda ffc, st=st: ht8[:, ffc, c8[st] : c8[st + 1]],
                    CH8[st],
                )

            # mid/f16-tier streams launch behind the fp8 weights; the fp8 mm2
            # below only needs w2q, which is already queued.
            xt0 = xpool.tile([P, KT1, 512], f16, tag="xt")
            nc.sync.dma_start(xt0[:], xT16_d[:, nst16])  # mid supertile 0
            w1sl0 = []
            for kt in range(2):
                t = w1pool.tile([P, KT1, W1SL], f16, tag="w1sl")
                nc.sync.dma_start(t[:], w1h_d[:, kt])
                w1sl0.append(t)
            w2h_sb = wpool.tile([P, KT2, D_MODEL], f16, tag="w2h")
            for q in range(4):
                nc.sync.dma_start(
                    w2h_sb[:, q * 8 : (q + 1) * 8, :], w2h_d[:, q * 8 : (q + 1) * 8, :]
                )

            for st in range(nst8):
                mm2_fp8(
                    ht8, c8[st], CH8[st],
                    N16 + NM + c8[st], NM // P + c8[st] // P,
                )

            # ---------- mid tier: f16 mm1 -> fp8 h -> DR mm2 ----------
            tokm = 0
            for stm, tok_len in enumerate(CHM):
                if stm == 0:
                    xt = xt0
                    w1sl = list(w1sl0)
                else:
                    xt = xpool.tile([P, KT1, 512], f16, tag="xt")
                    nc.sync.dma_start(xt[:], xT16_d[:, nst16 + stm])
                    w1sl = []
                for sl in range(len(w1sl), N_SL):
                    t = w1pool.tile([P, KT1, W1SL], f16, tag="w1sl")
                    nc.sync.dma_start(t[:], w1h_d[:, sl])
                    w1sl.append(t)
                htm = hpool.tile([P, KT2, 512], f16, tag="ht")
                htm8 = htm[:].bitcast(f8)  # [P, KT2, 1024] view
                col0 = 512 * (stm % 2)
                mm1_f16(
                    xt, w1sl,
                    lambda ffc: htm8[:, ffc, col0 : col0 + tok_len],
                    tok_len, center=True,
                )
                mm2_fp8(htm8, col0, tok_len, N16 + tokm, tokm // P)
                tokm += tok_len

            # ---------- f16 tier ----------
            tok0 = 0
            for st, tok_len in enumerate(CH16):
                xt = xpool.tile([P, KT1, 512], f16, tag="xt")
                nc.sync.dma_start(xt[:], xT16_d[:, st])
                w1sl = []
                for sl in range(N_SL):
                    t = w1pool.tile([P, KT1, W1SL], f16, tag="w1sl")
                    nc.sync.dma_start(t[:], w1h_d[:, sl])
                    w1sl.append(t)
                ht = hpool.tile([P, KT2, 512], f16, tag="ht")
                mm1_f16(xt, w1sl, lambda ffc: ht[:, ffc, :tok_len], tok_len)
                mm2_f16(ht, tok_len, tok0, tok0 // P, drain_split=(st == nst16 - 1))
                tok0 += tok_len
    nc.compile()
    return nc


def _get_nc(act="Gelu"):
    if act not in _NC_CACHE:
        _NC_CACHE[act] = _build(act)
    return _NC_CACHE[act]


def _perm_w(w, kt, cols):
    """[kt*P, cols] -> [P, kt, cols] with row = kt*P + p."""
    return np.ascontiguousarray(w.reshape(kt, P, cols).transpose(1, 0, 2))


def _perm_w_sliced(w, kt, n_sl, slw):
    """[kt*P, n_sl*slw] -> [P, n_sl, kt, slw]."""
    return np.ascontiguousarray(w.reshape(kt, P, n_sl, slw).transpose(1, 2, 0, 3))


def _pack_x(xcols, chunks, dtype):
    """[D_MODEL, n] columns -> [P, n_st, KT1, 512] supertile blocks."""
    out = np.zeros((P, len(chunks), KT1, 512), dtype=dtype)
    t0 = 0
    for st, L in enumerate(chunks):
        blk = xcols[:, t0 : t0 + L].reshape(KT1, P, L).transpose(1, 0, 2)
        out[:, st, :, :L] = blk
        t0 += L
    return out


def _erf(v):
    try:
        from scipy.special import erf
        return erf(v)
    except ImportError:
        # Abramowitz & Stegun 7.1.26 (|err| < 1.5e-7), numpy-only fallback.
        s = np.sign(v)
        a = np.abs(v)
        t = 1.0 / (1.0 + 0.3275911 * a)
        y = 1.0 - (
            ((((1.061405429 * t - 1.453152027) * t) + 1.421413741) * t
             - 0.284496736) * t + 0.254829592
        ) * t * np.exp(-a * a)
        return s * y


def _gelu(v):
    return v * 0.5 * (1.0 + _erf(v / np.sqrt(2.0)))


def kernel(x, router_w, router_b, w1, b1, w2, b2):
    from concourse.bass_utils import run_bass_kernel_spmd

    x = np.asarray(x, dtype=np.float32)
    router_w = np.asarray(router_w, dtype=np.float32)
    router_b = np.asarray(router_b, dtype=np.float32)
    w1 = np.asarray(w1, dtype=np.float32)
    b1 = np.asarray(b1, dtype=np.float32)
    w2 = np.asarray(w2, dtype=np.float32)
    b2 = np.asarray(b2, dtype=np.float32)

    B, S, D = x.shape
    T = B * S
    xf = x.reshape(T, D)

    # --- host router: top-2 + softmax gates (tiny: T x D x 8) ---
    logits = xf @ router_w + router_b                      # [T, 8] fp32
    sel0 = np.argmax(logits, axis=1)
    l0 = logits[np.arange(T), sel0]
    masked = logits.copy()
    masked[np.arange(T), sel0] = -np.inf
    sel1 = np.argmax(masked, axis=1)
    l1 = masked[np.arange(T), sel1]
    e1 = np.exp(l1 - l0)
    g0 = 1.0 / (1.0 + e1)
    g1 = e1 / (1.0 + e1)

    # --- dispatch: per expert, rank pairs by gate; tier by rank.
    # Device slots cover ranks [0, SLOTS) in order f16 | mid | fp8; ranks
    # beyond SLOTS (capacity overflow) are computed exactly on the host.
    tiers = []  # per expert: (ids, g) rank-sorted
    for e in range(NUM_EXPERTS):
        ids0 = np.nonzero(sel0 == e)[0]
        ids1 = np.nonzero(sel1 == e)[0]
        ids = np.concatenate([ids0, ids1])
        g = np.concatenate([g0[ids0], g1[ids1]]).astype(np.float32)
        order = np.argsort(-g, kind="stable")
        tiers.append((ids[order], g[order]))

    nc = _get_nc()

    in_maps = []
    for e in range(NUM_EXPERTS):
        ids, g = tiers[e]
        ids_dev, g_dev = ids[:SLOTS], g[:SLOTS]
        n_dev = len(ids_dev)
        x16 = np.zeros((D_MODEL, N16 + NM), dtype=np.float32)
        x16[:, : min(n_dev, N16 + NM)] = xf[ids_dev[: N16 + NM]].T
        x8 = np.zeros((D_MODEL, N8), dtype=np.float32)
        if n_dev > N16 + NM:
            x8[:, : n_dev - (N16 + NM)] = xf[ids_dev[N16 + NM :]].T
        gp = np.zeros((SLOTS,), dtype=np.float32)
        gp[:n_dev] = g_dev
        # rows whose mm2 goes through w2q: psum = (SH*(h-HC)) @ (S2*w2)
        gp[N16:] /= S2 * SH
        in_maps.append(
            {
                "xT16": _pack_x(x16.astype(F16), CH16 + CHM, F16),
                "xT8": _pack_x(
                    np.clip(x8 * SX, -240.0, 240.0).astype(F8), CH8, F8
                ),
                "w1h": _perm_w_sliced(w1[e].astype(F16), KT1, N_SL, W1SL),
                "w1q": _perm_w_sliced((w1[e] * S1).astype(F8), KT1, N_QSL, W1QSL),
                "w2h": _perm_w(w2[e].astype(F16), KT2, D_MODEL),
                "w2q": _perm_w((w2[e] * S2).astype(F8), KT2, D_MODEL),
                "b1": np.ascontiguousarray(b1[e].reshape(FFC, P).T),
                "g16": np.ascontiguousarray(gp[:N16].reshape(N16 // P, P).T),
                "gq": np.ascontiguousarray(gp[N16:].reshape(NQ // P, P).T),
            }
        )

    try:
        res = run_bass_kernel_spmd(nc, in_maps, core_ids=list(range(NUM_EXPERTS)))
    except Exception:
        # Transient device errors (e.g. NRT_EXEC_UNIT_UNRECOVERABLE from a
        # wedged core) usually clear on a fresh attempt.
        res = run_bass_kernel_spmd(nc, in_maps, core_ids=list(range(NUM_EXPERTS)))
    global LAST_RESULTS
    LAST_RESULTS = res

    out = np.zeros((T, D), dtype=np.float32)
    for e in range(NUM_EXPERTS):
        ids, g = tiers[e]
        n_dev = min(len(ids), SLOTS)
        r = res.results[e]["out"]
        out[ids[:n_dev]] += r[:n_dev]
        # add back the centering constant: h@w2q = (h-HC)@w2q + HC*colsum(w2q)
        w2q_deq = (w2[e] * S2).astype(F8).astype(np.float32) / S2
        corr = HC * w2q_deq.sum(axis=0)
        out[ids[N16:n_dev]] += np.outer(g[N16:n_dev], corr)
        if len(ids) > SLOTS:
            # capacity overflow: exact fp32 FFN on host for the tail
            ho_ids = ids[SLOTS:]
            h = _gelu(xf[ho_ids] @ w1[e] + b1[e])
            out[ho_ids] += (h @ w2[e]) * g[SLOTS:][:, None]
    if b2.any():
        out += g0[:, None] * b2[sel0] + g1[:, None] * b2[sel1]
    return out.reshape(B, S, D)


# revision 39
# speedup vs baseline: 1.0205x; 1.0030x over previous
"""MoE layer (8 experts, top-2) Trainium2 kernel — fp16/fp8 tiers + capacity.

Expert-parallel (per sharding hint): host computes the tiny router
(logits -> top-2 -> softmax gates) and dispatches tokens to the 8 NeuronCores
by selected expert; core e runs expert e's FFN.  Each expert's token-pairs
are ranked by gate and split into fixed-size tiers (same sizes on every core
so one NEFF serves all 8 SPMD):

  f16 tier (N16=2048, largest gates)  fp16 mm1 + fp16 mm2           cost 1.00
  mid tier (NM=896)                   fp16 mm1 + fp8 DoubleRow mm2  cost 0.75
  fp8 tier (N8=512, smallest gates)   fp8 DR mm1 + fp8 DR mm2       cost 0.50
  beyond SLOTS=3456 (expert capacity) exact fp32 on host (small tail)

The mid tier quantizes only the mm2 inputs (h, w2), sitting slightly below
the f16<->fp8 cost/error^2 mixing chord.  h entering fp8 mm2 is centered
(h - 0.28125, its mean) and scaled x32 before e4m3 quantization — DVE does
the affine from an f16 staging tile, and the constant term is added back
exactly on the host — cutting the h quantization error ~19%.  Tier sizes
were tuned against a host-side ml_dtypes simulator that predicts the
measured HW rel-err to ~1e-4 (target ~0.0195 of the 2e-2 budget).

fp8 (e4m3) DoubleRow matmuls run 2x the fp16 PE rate.  All fp8 operands are
scaled into e4m3's normal range (x*16, w1*64, w2*128) to avoid subnormal
quantization loss; the scales are undone exactly via the activation's
input scale (1/1024, power of two) and host-side gate scaling (g/128).
fp16 (not bf16) for everything else makes the high tier's quantization
error negligible, freeing the 2e-2 rel-err budget for bigger fp8 tiers.
Expert overflow past SLOTS (capacity factor SLOTS/4096) is computed exactly
on the host (tens of GFLOP, sub-second) — standard MoE capacity handling,
with recompute instead of token dropping.

Execution order puts the full-fp8 tier FIRST: its PE work needs only ~9MB
of DMA (fp8 weights + tokens), hiding the cold-start weight load; w1q is
split into 8 per-slice tiles so the first matmul only waits on slice 0.
The mid tier runs second (streams w1h, needs only w2q for mm2); the f16
tier last (needs w2h, which loads during the earlier phases).

SBUF keeps w2h(f16) + w1q/w2q(fp8) resident; w1h streams per supertile.
All streamed tensors are pre-permuted on the host into SBUF consumption
order so each DMA moves >=4KB contiguous per partition.  fp8 x tiles alias
the f16 tiles' storage via bitcast.  All matmuls accumulate in fp32 PSUM.
"""

import numpy as np
import ml_dtypes

P = 128
D_MODEL = 1024
D_FF = 4096
NUM_EXPERTS = 8
KT1 = D_MODEL // P   # 8  k-tiles for mm1
KT2 = D_FF // P      # 32 k-tiles for mm2
FFC = D_FF // P      # 32 ff chunks (mm1 output partition tiles)
NH = D_MODEL // 2    # 512, free dim of mm2 matmuls
W1SL = 256           # ff columns per streamed f16-w1 slice
N_SL = D_FF // W1SL  # 16 slices per supertile
W1QSL = 512          # ff columns per fp8-w1 slice
N_QSL = D_FF // W1QSL

N16 = 2048           # f16 tier slots per expert
NM = 896             # mid tier (f16 mm1 + fp8 mm2)
N8 = 512             # full-fp8 tier
NQ = NM + N8         # rows whose mm2 goes through w2q (gates get /(S2*SH))
SLOTS = N16 + NM + N8  # 3456 = expert capacity; overflow -> exact host

SX = 16.0            # x scale into e4m3
S1 = 64.0            # w1 scale into e4m3
S2 = 128.0           # w2 scale into e4m3 (undone via gates)
ACT_SCALE = 1.0 / (SX * S1)
# h fed to fp8 mm2 is centered then scaled: h8 = e4m3(SH*(h - HC)).  Centering
# shrinks |h - HC| (h = gelu(z) has mean ~0.28), cutting the h quantization
# error ~19%; the constant term HC @ w2q is added back exactly on the host.
SH = 32.0
HC = 0.28125         # == 9/32, so SH*HC = 9 exactly

F16 = np.float16
F8 = ml_dtypes.float8_e4m3

_NC_CACHE: dict = {}
LAST_RESULTS = None  # BassKernelResults of the most recent device run

# Chunk sizes are chosen so weight loads hide behind the stream: fp8 DR
# matmuls are LDWEIGHTS-bound below 512-wide streams, f16 below 256-wide.
CH16 = [512, 512, 512, 512]        # sums to N16
CHM = [512, 384]                   # sums to NM
CH8 = [512]                        # sums to N8
assert sum(CH16) == N16 and sum(CHM) == NM and sum(CH8) == N8


def _build(act="Gelu"):
    import concourse.bacc as bacc
    import concourse.tile as tile
    import concourse.mybir as mybir

    f16 = mybir.dt.float16
    f8 = mybir.dt.float8e4
    f32 = mybir.dt.float32
    DR = mybir.MatmulPerfMode.DoubleRow
    GELU = getattr(mybir.ActivationFunctionType, act)

    nst16 = len(CH16)
    nstm = len(CHM)
    nst8 = len(CH8)

    nc = bacc.Bacc("TRN2", target_bir_lowering=False, debug=False)
    # All inputs are host-side pre-permuted into [partition, ...consumption
    # order...] so every DMA line is long and contiguous.  xT16 holds the
    # f16-tier supertiles then the mid tier's; xT8 the full-fp8 tier's.
    # Device out rows are in rank order: f16 | mid | fp8.
    xT16_d = nc.dram_tensor("xT16", [P, nst16 + nstm, KT1, 512], f16, kind="ExternalInput").ap()
    xT8_d = nc.dram_tensor("xT8", [P, nst8, KT1, 512], f8, kind="ExternalInput").ap()
    w1h_d = nc.dram_tensor("w1h", [P, N_SL, KT1, W1SL], f16, kind="ExternalInput").ap()
    w1q_d = nc.dram_tensor("w1q", [P, N_QSL, KT1, W1QSL], f8, kind="ExternalInput").ap()
    w2h_d = nc.dram_tensor("w2h", [P, KT2, D_MODEL], f16, kind="ExternalInput").ap()
    w2q_d = nc.dram_tensor("w2q", [P, KT2, D_MODEL], f8, kind="ExternalInput").ap()
    b1_d = nc.dram_tensor("b1", [P, FFC], f32, kind="ExternalInput").ap()
    g16_d = nc.dram_tensor("g16", [P, N16 // P], f32, kind="ExternalInput").ap()
    gq_d = nc.dram_tensor("gq", [P, NQ // P], f32, kind="ExternalInput").ap()
    out_d = nc.dram_tensor("out", [SLOTS, D_MODEL], f16, kind="ExternalOutput").ap()

    with tile.TileContext(nc) as tc:
        with (
            tc.tile_pool(name="wpool", bufs=1) as wpool,
            tc.tile_pool(name="w1pool", bufs=5) as w1pool,
            tc.tile_pool(name="xpool", bufs=2) as xpool,
            tc.tile_pool(name="hpool", bufs=1) as hpool,
            tc.tile_pool(name="hsp", bufs=2) as hsp,
            tc.tile_pool(name="opool", bufs=3) as opool,
            tc.tile_pool(name="php", bufs=4, space="PSUM") as php,
            tc.tile_pool(name="pyp", bufs=4, space="PSUM") as pyp,
        ):
            # --- prologue: cold-start DMA is per-transfer-latency-bound, so
            # the first matmul's dependencies (x8 supertile + w1q slice 0) are
            # few fat transfers issued ahead of everything else.  w1q lives in
            # 8 per-slice tiles so matmuls unlock slice by slice.
            # The first matmul needs only x k-tiles 0-3 and w1q slice-0 cols
            # 0-255; splitting those transfers halves the cold-DMA bytes the
            # PE waits on (Tile tracks sub-tile write regions).
            xt8s = []
            for st in range(nst8):
                xtA = xpool.tile([P, KT1, 512], f16, tag="xt")
                xt8s.append(xtA[:].bitcast(f8))
                if st == 0:
                    nc.sync.dma_start(
                        xt8s[st][:, : KT1 // 2, :512], xT8_d[:, st, : KT1 // 2]
                    )
                else:
                    nc.sync.dma_start(xt8s[st][:, :, :512], xT8_d[:, st])
            w1q_sl = []
            for sl in range(N_QSL):
                t = wpool.tile([P, KT1, W1QSL], f8, tag=f"w1q{sl}")
                if sl == 0:
                    nc.sync.dma_start(t[:, :, :256], w1q_d[:, 0, :, :256])
                    nc.sync.dma_start(
                        xt8s[0][:, KT1 // 2 :, :512], xT8_d[:, 0, KT1 // 2 :]
                    )
                    nc.sync.dma_start(t[:, :, 256:], w1q_d[:, 0, :, 256:])
                    # b1 gates the first activation (and through PSUM reuse,
                    # the 5th ffc's matmuls) — land it right after slice 0.
                    b1_sb = wpool.tile([P, FFC], f32, tag="b1")
                    nc.sync.dma_start(b1_sb[:], b1_d[:])
                else:
                    nc.sync.dma_start(t[:], w1q_d[:, sl])
                w1q_sl.append(t)
            gq_sb = wpool.tile([P, NQ // P], f32, tag="gq")
            nc.sync.dma_start(gq_sb[:], gq_d[:])
            g16_sb = wpool.tile([P, N16 // P], f32, tag="g16")
            nc.sync.dma_start(g16_sb[:], g16_d[:])
            w2q_sb = wpool.tile([P, KT2, D_MODEL], f8, tag="w2q")
            for q in range(4):
                nc.sync.dma_start(
                    w2q_sb[:, q * 8 : (q + 1) * 8, :], w2q_d[:, q * 8 : (q + 1) * 8, :]
                )
            def center_h(dst_view, ph, tok_len, ffc, act_scale):
                """GELU to f16 staging, then DVE affine SH*(h-HC) into fp8."""
                hs = hsp.tile([P, 512], f16, tag="hs")
                nc.scalar.activation(
                    hs[:, :tok_len], ph[:, :tok_len], GELU,
                    bias=b1_sb[:, ffc : ffc + 1], scale=act_scale,
                )
                nc.vector.tensor_scalar(
                    dst_view, hs[:, :tok_len], SH, -SH * HC,
                    op0=mybir.AluOpType.mult, op1=mybir.AluOpType.add,
                )

            # --- HAM warmup: the PE clock sits at 1.2 GHz until ~3.4us of
            # sustained matmul activity.  These zero matmuls need no DMA, so
            # they run while the first transfers are still in flight and the
            # real stream starts at full clock.  (PE is idle here anyway.)
            zw = wpool.tile([P, P], f16, tag="zw")
            zx = wpool.tile([P, 512], f16, tag="zx")
            nc.vector.memset(zw[:], 0.0)
            nc.vector.memset(zx[:], 0.0)
            for _ in range(10):
                pz = php.tile([P, 512], f32, tag="ph")
                nc.tensor.matmul(pz[:], zw[:], zx[:], start=True, stop=True)

            def mm1_fp8(xt8, ht_ffc_view, tok_len):
                """DR mm1 on scaled fp8; activation undoes the x/w1 scales."""
                for ffc in range(FFC):
                    ph = php.tile([P, 512], f32, tag="ph")
                    sl, col = divmod(ffc * P, W1QSL)
                    for k2 in range(KT1 // 2):
                        nc.tensor.matmul(
                            ph[:, :tok_len],
                            w1q_sl[sl][:, 2 * k2 : 2 * k2 + 2, col : col + P],
                            xt8[:, 2 * k2 : 2 * k2 + 2, :tok_len],
                            start=(k2 == 0),
                            stop=(k2 == KT1 // 2 - 1),
                            perf_mode=DR,
                        )
                    center_h(ht_ffc_view(ffc), ph, tok_len, ffc, ACT_SCALE)

            def mm1_f16(xt, w1sl, ht_ffc_view, tok_len, center=False):
                for ffc in range(FFC):
                    ph = php.tile([P, 512], f32, tag="ph")
                    sl, col = divmod(ffc * P, W1SL)
                    for kt in range(KT1):
                        nc.tensor.matmul(
                            ph[:, :tok_len],
                            w1sl[sl][:, kt, col : col + P],
                            xt[:, kt, :tok_len],
                            start=(kt == 0),
                            stop=(kt == KT1 - 1),
                        )
                    if center:
                        center_h(ht_ffc_view(ffc), ph, tok_len, ffc, 1.0)
                    else:
                        nc.scalar.activation(
                            ht_ffc_view(ffc), ph[:, :tok_len], GELU,
                            bias=b1_sb[:, ffc : ffc + 1], scale=1.0,
                        )

            def mm2_fp8(ht8, col0, tok_len, out_row0, gcol0, drain_split=False):
                """DR mm2 through scaled w2q; gates (pre-divided by S2) undo it."""
                n_mt = tok_len // P
                for mt in range(n_mt):
                    if drain_split and mt == n_mt - 1:
                        for nb in (0, 1):
                            py = pyp.tile([P, NH], f32, tag="py")
                            for k2 in range(KT2 // 2):
                                nc.tensor.matmul(
                                    py,
                                    ht8[:, 2 * k2 : 2 * k2 + 2, col0 + mt * P : col0 + (mt + 1) * P],
                                    w2q_sb[:, 2 * k2 : 2 * k2 + 2, nb * NH : (nb + 1) * NH],
                                    start=(k2 == 0), stop=(k2 == KT2 // 2 - 1),
                                    perf_mode=DR,
                                )
                            ot = opool.tile([P, NH], f16, tag="ot")
                            nc.vector.tensor_scalar_mul(
                                ot[:], py[:], gq_sb[:, gcol0 + mt : gcol0 + mt + 1]
                            )
                            nc.sync.dma_start(
                                out_d[
                                    out_row0 + mt * P : out_row0 + (mt + 1) * P,
                                    nb * NH : (nb + 1) * NH,
                                ],
                                ot[:],
                            )
                        continue
                    py0 = pyp.tile([P, NH], f32, tag="py")
                    py1 = pyp.tile([P, NH], f32, tag="py")
                    for k2 in range(KT2 // 2):
                        lhsT = ht8[:, 2 * k2 : 2 * k2 + 2, col0 + mt * P : col0 + (mt + 1) * P]
                        nc.tensor.matmul(
                            py0, lhsT, w2q_sb[:, 2 * k2 : 2 * k2 + 2, 0:NH],
                            start=(k2 == 0), stop=(k2 == KT2 // 2 - 1), perf_mode=DR,
                        )
                        nc.tensor.matmul(
                            py1, lhsT, w2q_sb[:, 2 * k2 : 2 * k2 + 2, NH:D_MODEL],
                            start=(k2 == 0), stop=(k2 == KT2 // 2 - 1), perf_mode=DR,
                        )
                    for nb, py in ((0, py0), (1, py1)):
                        ot = opool.tile([P, NH], f16, tag="ot")
                        nc.vector.tensor_scalar_mul(
                            ot[:], py[:], gq_sb[:, gcol0 + mt : gcol0 + mt + 1]
                        )
                        nc.sync.dma_start(
                            out_d[
                                out_row0 + mt * P : out_row0 + (mt + 1) * P,
                                nb * NH : (nb + 1) * NH,
                            ],
                            ot[:],
                        )

            def mm2_f16(ht, tok_len, out_row0, gcol0, drain_split=False):
                n_mt = tok_len // P
                for mt in range(n_mt):
                    if drain_split and mt == n_mt - 1:
                        # Kernel-final block: run four quarter-width output
                        # pieces sequentially so each piece's gate-mult +
                        # store overlaps the next piece's matmuls; only the
                        # last 256-col chain trails the final matmul.
                        NQW = D_MODEL // 4
                        for nb in range(4):
                            py = pyp.tile([P, NH], f32, tag="py")
                            for kt in range(KT2):
                                nc.tensor.matmul(
                                    py[:, :NQW], ht[:, kt, mt * P : (mt + 1) * P],
                                    w2h_sb[:, kt, nb * NQW : (nb + 1) * NQW],
                                    start=(kt == 0), stop=(kt == KT2 - 1),
                                )
                            ot = opool.tile([P, NH], f16, tag="ot")
                            nc.vector.tensor_scalar_mul(
                                ot[:, :NQW], py[:, :NQW],
                                g16_sb[:, gcol0 + mt : gcol0 + mt + 1],
                            )
                            nc.sync.dma_start(
                                out_d[
                                    out_row0 + mt * P : out_row0 + (mt + 1) * P,
                                    nb * NQW : (nb + 1) * NQW,
                                ],
                                ot[:, :NQW],
                            )
                        continue
                    py0 = pyp.tile([P, NH], f32, tag="py")
                    py1 = pyp.tile([P, NH], f32, tag="py")
                    for kt in range(KT2):
                        lhsT = ht[:, kt, mt * P : (mt + 1) * P]
                        nc.tensor.matmul(
                            py0, lhsT, w2h_sb[:, kt, 0:NH],
                            start=(kt == 0), stop=(kt == KT2 - 1),
                        )
                        nc.tensor.matmul(
                            py1, lhsT, w2h_sb[:, kt, NH:D_MODEL],
                            start=(kt == 0), stop=(kt == KT2 - 1),
                        )
                    for nb, py in ((0, py0), (1, py1)):
                        ot = opool.tile([P, NH], f16, tag="ot")
                        nc.vector.tensor_scalar_mul(
                            ot[:], py[:], g16_sb[:, gcol0 + mt : gcol0 + mt + 1]
                        )
                        nc.sync.dma_start(
                            out_d[
                                out_row0 + mt * P : out_row0 + (mt + 1) * P,
                                nb * NH : (nb + 1) * NH,
                            ],
                            ot[:],
                        )

            # ---------- full-fp8 tier (DMA-cheap warmup) ----------
            # All supertiles' mm1 run back-to-back into disjoint column
            # ranges of one h tile; their mm2 follow.
            ht_f8 = hpool.tile([P, KT2, 512], f16, tag="ht")
            ht8 = ht_f8[:].bitcast(f8)  # [P, KT2, 1024] view
            c8 = [0]
            for L in CH8:
                c8.append(c8[-1] + L)
            for st in range(nst8):
                mm1_fp8(
                    xt8s[st],
                    lambda ffc, st=st: ht8[:, ffc, c8[st] : c8[st + 1]],
                    CH8[st],
                )

            # mid/f16-tier streams launch behind the fp8 weights; the fp8 mm2
            # below only needs w2q, which is already queued.
            xt0 = xpool.tile([P, KT1, 512], f16, tag="xt")
            nc.sync.dma_start(xt0[:], xT16_d[:, nst16])  # mid supertile 0
            w1sl0 = []
            for kt in range(2):
                t = w1pool.tile([P, KT1, W1SL], f16, tag="w1sl")
                nc.sync.dma_start(t[:], w1h_d[:, kt])
                w1sl0.append(t)
            w2h_sb = wpool.tile([P, KT2, D_MODEL], f16, tag="w2h")
            for q in range(4):
                nc.sync.dma_start(
                    w2h_sb[:, q * 8 : (q + 1) * 8, :], w2h_d[:, q * 8 : (q + 1) * 8, :]
                )

            for st in range(nst8):
                mm2_fp8(
                    ht8, c8[st], CH8[st],
                    N16 + NM + c8[st], NM // P + c8[st] // P,
                )

            # ---------- mid tier: f16 mm1 -> fp8 h -> DR mm2 ----------
            tokm = 0
            for stm, tok_len in enumerate(CHM):
                if stm == 0:
                    xt = xt0
                    w1sl = list(w1sl0)
                else:
                    xt = xpool.tile([P, KT1, 512], f16, tag="xt")
                    nc.sync.dma_start(xt[:], xT16_d[:, nst16 + stm])
                    w1sl = []
                for sl in range(len(w1sl), N_SL):
                    t = w1pool.tile([P, KT1, W1SL], f16, tag="w1sl")
                    nc.sync.dma_start(t[:], w1h_d[:, sl])
                    w1sl.append(t)
                htm = hpool.tile([P, KT2, 512], f16, tag="ht")
                htm8 = htm[:].bitcast(f8)  # [P, KT2, 1024] view
                col0 = 512 * (stm % 2)
                mm1_f16(
                    xt, w1sl,
                    lambda ffc: htm8[:, ffc, col0 : col0 + tok_len],
                    tok_len, center=True,
                )
                mm2_fp8(htm8, col0, tok_len, N16 + tokm, tokm // P)
                tokm += tok_len

            # ---------- f16 tier ----------
            tok0 = 0
            for st, tok_len in enumerate(CH16):
                xt = xpool.tile([P, KT1, 512], f16, tag="xt")
                nc.sync.dma_start(xt[:], xT16_d[:, st])
                w1sl = []
                for sl in range(N_SL):
                    t = w1pool.tile([P, KT1, W1SL], f16, tag="w1sl")
                    nc.sync.dma_start(t[:], w1h_d[:, sl])
                    w1sl.append(t)
                ht = hpool.tile([P, KT2, 512], f16, tag="ht")
                mm1_f16(xt, w1sl, lambda ffc: ht[:, ffc, :tok_len], tok_len)
                mm2_f16(ht, tok_len, tok0, tok0 // P, drain_split=(st == nst16 - 1))
                tok0 += tok_len
    nc.compile()
    return nc


def _get_nc(act="Gelu"):
    if act not in _NC_CACHE:
        _NC_CACHE[act] = _build(act)
    return _NC_CACHE[act]


def _perm_w(w, kt, cols):
    """[kt*P, cols] -> [P, kt, cols] with row = kt*P + p."""
    return np.ascontiguousarray(w.reshape(kt, P, cols).transpose(1, 0, 2))


def _perm_w_sliced(w, kt, n_sl, slw):
    """[kt*P, n_sl*slw] -> [P, n_sl, kt, slw]."""
    return np.ascontiguousarray(w.reshape(kt, P, n_sl, slw).transpose(1, 2, 0, 3))


def _pack_x(xcols, chunks, dtype):
    """[D_MODEL, n] columns -> [P, n_st, KT1, 512] supertile blocks."""
    out = np.zeros((P, len(chunks), KT1, 512), dtype=dtype)
    t0 = 0
    for st, L in enumerate(chunks):
        blk = xcols[:, t0 : t0 + L].reshape(KT1, P, L).transpose(1, 0, 2)
        out[:, st, :, :L] = blk
        t0 += L
    return out


def _erf(v):
    try:
        from scipy.special import erf
        return erf(v)
    except ImportError:
        # Abramowitz & Stegun 7.1.26 (|err| < 1.5e-7), numpy-only fallback.
        s = np.sign(v)
        a = np.abs(v)
        t = 1.0 / (1.0 + 0.3275911 * a)
        y = 1.0 - (
            ((((1.061405429 * t - 1.453152027) * t) + 1.421413741) * t
             - 0.284496736) * t + 0.254829592
        ) * t * np.exp(-a * a)
        return s * y


def _gelu(v):
    return v * 0.5 * (1.0 + _erf(v / np.sqrt(2.0)))


def kernel(x, router_w, router_b, w1, b1, w2, b2):
    from concourse.bass_utils import run_bass_kernel_spmd

    x = np.asarray(x, dtype=np.float32)
    router_w = np.asarray(router_w, dtype=np.float32)
    router_b = np.asarray(router_b, dtype=np.float32)
    w1 = np.asarray(w1, dtype=np.float32)
    b1 = np.asarray(b1, dtype=np.float32)
    w2 = np.asarray(w2, dtype=np.float32)
    b2 = np.asarray(b2, dtype=np.float32)

    B, S, D = x.shape
    T = B * S
    xf = x.reshape(T, D)

    # --- host router: top-2 + softmax gates (tiny: T x D x 8) ---
    logits = xf @ router_w + router_b                      # [T, 8] fp32
    sel0 = np.argmax(logits, axis=1)
    l0 = logits[np.arange(T), sel0]
    masked = logits.copy()
    masked[np.arange(T), sel0] = -np.inf
    sel1 = np.argmax(masked, axis=1)
    l1 = masked[np.arange(T), sel1]
    e1 = np.exp(l1 - l0)
    g0 = 1.0 / (1.0 + e1)
    g1 = e1 / (1.0 + e1)

    # --- dispatch: per expert, rank pairs by gate; tier by rank.
    # Device slots cover ranks [0, SLOTS) in order f16 | mid | fp8; ranks
    # beyond SLOTS (capacity overflow) are computed exactly on the host.
    tiers = []  # per expert: (ids, g) rank-sorted
    for e in range(NUM_EXPERTS):
        ids0 = np.nonzero(sel0 == e)[0]
        ids1 = np.nonzero(sel1 == e)[0]
        ids = np.concatenate([ids0, ids1])
        g = np.concatenate([g0[ids0], g1[ids1]]).astype(np.float32)
        order = np.argsort(-g, kind="stable")
        tiers.append((ids[order], g[order]))

    nc = _get_nc()

    in_maps = []
    for e in range(NUM_EXPERTS):
        ids, g = tiers[e]
        ids_dev, g_dev = ids[:SLOTS], g[:SLOTS]
        n_dev = len(ids_dev)
        x16 = np.zeros((D_MODEL, N16 + NM), dtype=np.float32)
        x16[:, : min(n_dev, N16 + NM)] = xf[ids_dev[: N16 + NM]].T
        x8 = np.zeros((D_MODEL, N8), dtype=np.float32)
        if n_dev > N16 + NM:
            x8[:, : n_dev - (N16 + NM)] = xf[ids_dev[N16 + NM :]].T
        gp = np.zeros((SLOTS,), dtype=np.float32)
        gp[:n_dev] = g_dev
        # rows whose mm2 goes through w2q: psum = (SH*(h-HC)) @ (S2*w2)
        gp[N16:] /= S2 * SH
        in_maps.append(
            {
                "xT16": _pack_x(x16.astype(F16), CH16 + CHM, F16),
                "xT8": _pack_x(
                    np.clip(x8 * SX, -240.0, 240.0).astype(F8), CH8, F8
                ),
                "w1h": _perm_w_sliced(w1[e].astype(F16), KT1, N_SL, W1SL),
                "w1q": _perm_w_sliced((w1[e] * S1).astype(F8), KT1, N_QSL, W1QSL),
                "w2h": _perm_w(w2[e].astype(F16), KT2, D_MODEL),
                "w2q": _perm_w((w2[e] * S2).astype(F8), KT2, D_MODEL),
                "b1": np.ascontiguousarray(b1[e].reshape(FFC, P).T),
                "g16": np.ascontiguousarray(gp[:N16].reshape(N16 // P, P).T),
                "gq": np.ascontiguousarray(gp[N16:].reshape(NQ // P, P).T),
            }
        )

    try:
        res = run_bass_kernel_spmd(nc, in_maps, core_ids=list(range(NUM_EXPERTS)))
    except Exception:
        # Transient device errors (e.g. NRT_EXEC_UNIT_UNRECOVERABLE from a
        # wedged core) usually clear on a fresh attempt.
        res = run_bass_kernel_spmd(nc, in_maps, core_ids=list(range(NUM_EXPERTS)))
    global LAST_RESULTS
    LAST_RESULTS = res

    out = np.zeros((T, D), dtype=np.float32)
    for e in range(NUM_EXPERTS):
        ids, g = tiers[e]
        n_dev = min(len(ids), SLOTS)
        r = res.results[e]["out"]
        out[ids[:n_dev]] += r[:n_dev]
        # add back the centering constant: h@w2q = (h-HC)@w2q + HC*colsum(w2q)
        w2q_deq = (w2[e] * S2).astype(F8).astype(np.float32) / S2
        corr = HC * w2q_deq.sum(axis=0)
        out[ids[N16:n_dev]] += np.outer(g[N16:n_dev], corr)
        if len(ids) > SLOTS:
            # capacity overflow: exact fp32 FFN on host for the tail
            ho_ids = ids[SLOTS:]
            h = _gelu(xf[ho_ids] @ w1[e] + b1[e])
            out[ho_ids] += (h @ w2[e]) * g[SLOTS:][:, None]
    if b2.any():
        out += g0[:, None] * b2[sel0] + g1[:, None] * b2[sel1]
    return out.reshape(B, S, D)
